# revision 67
# baseline (speedup 1.0000x reference)
"""Distributed attention kernel for Trainium2 (8 NeuronCores).

Problem: nn_Attention (B=8, S=2048, d_model=512, d_hid=512, fp32).
Sharding: data-parallel over batch - one batch element per core, no
collectives. Each core computes softmax(Q K^T / sqrt(d)) V for its
[2048, 512] slice.

Per-core plan (matmul operands bf16 except where noted):
  * M-trick: scores = (x Wq)(x Wk)^T = x (Wq Wk^T) x^T. Precompute
    M = Wq Wk^T (8k PE cycles) and skip the K projection entirely
    (-32k cycles). qm = M^T x^T plays the role of Q^T; x^T itself
    plays the role of K^T. Bias handling stays exact: the Q-side bias
    terms are constant along the softmax axis (drop), the K-side term
    gamma[k] = x[k] . (Wk bq) is folded into the exp bias.
  * fp8 e4m3 + DoubleRow on half the AV k-chunks: 2x PE throughput on
    those chunks (-33k cycles); denominator computed from the
    quantized P so the correlated part of the quantization error
    cancels. Predicted rel-l2 ~1.75e-2 < 2e-2 gate.
  * Denominator in fp32: DVE tree-sum of the 16 P tiles, then one
    fp32 ones-matmul -> [1, 512]; transposes + DVE reciprocal give
    1/denom per-partition columns.
  * Software pipelining: AV(qb-1) is emitted after scores(qb), so the
    denom chain of a block hides under the next block's scores.
  * DMA with 8KB descriptors: x is loaded 4-consecutive-rows-per-
    partition; the output is assembled likewise by taking stride-4
    stationary slices of P in the AV matmul (o_blk[p, a, :] = row
    4p+a), then written with 4 partition-range dma_starts per block.
  * No max-subtraction in softmax: scores ~ N(0,1).
"""

import sys

for _p in ("/opt/trn_rl_repo",):
    if _p not in sys.path:
        sys.path.append(_p)

from contextlib import ExitStack

import numpy as np

import concourse.bass as bass
import concourse.mybir as mybir
import concourse.tile as tile
from concourse import bacc
from concourse.bass_utils import run_bass_kernel_spmd
from concourse.masks import make_identity

B = 8
S = 2048
D = 512
H = 512
P = 128
NB = 512  # matmul free-dim / PSUM bank (fp32)
FP = mybir.dt.float32
BF = mybir.dt.bfloat16
F8 = mybir.dt.float8e4
DR = mybir.MatmulPerfMode.DoubleRow
SCALE = 1.0 / float(np.sqrt(H))

D_CH = D // P   # 4 contraction chunks
S_T = S // P    # 16 sequence tiles
QB = S // NB    # 4 query blocks
N_BF_K = 4      # k-tiles 0..3 computed in bf16
N_F8_PAIRS = 6  # k-tiles 4..15 as 6 fp8 DoubleRow pairs
EXP = mybir.ActivationFunctionType.Exp
IDENT = mybir.ActivationFunctionType.Identity
# Constant subtracted from logits before exp (softmax-invariant): keeps
# exp(s) well under the e4m3 inf threshold (~240) for s up to ~7.5 sigma.
C_SHIFT = 2.0


DEBUG = False
_DBG = {}


def _build():
    nc = bacc.Bacc("TRN2", target_bir_lowering=False, debug=False)
    x = nc.dram_tensor("x", [S, D], FP, kind="ExternalInput").ap()
    wq = nc.dram_tensor("Wq", [D, H], FP, kind="ExternalInput").ap()
    bq = nc.dram_tensor("bq", [H], FP, kind="ExternalInput").ap()
    wk = nc.dram_tensor("Wk", [D, H], FP, kind="ExternalInput").ap()
    wv = nc.dram_tensor("Wv", [D, H], FP, kind="ExternalInput").ap()
    bv = nc.dram_tensor("bv", [H], FP, kind="ExternalInput").ap()
    out = nc.dram_tensor("out", [S, H], FP, kind="ExternalOutput").ap()
    if DEBUG:
        for nm, shp in (("dbg_xt0", [P, S]), ("dbg_qm0", [P, S]),
                        ("dbg_m0", [P, NB]), ("dbg_p00", [P, NB]),
                        ("dbg_p800", [P, NB]), ("dbg_rc00", [P, 1]),
                        ("dbg_gcol", [P, S_T]), ("dbg_ssum0", [P, NB]),
                        ("dbg_drow0", [1, NB]), ("dbg_drow0v", [1, NB]),
                        ("dbg_xt", [P, 4 * S]), ("dbg_qm", [P, 4 * S]),
                        ("dbg_pall", [P, S_T * NB]),
                        ("dbg_v0", [P, NB]), ("dbg_v80", [P, NB])):
            _DBG[nm] = nc.dram_tensor(nm, shp, FP, kind="ExternalOutput").ap()

    with tile.TileContext(nc, pool_alloc_mode="queue") as tc:
        _body(tc, x, wq, bq, wk, wv, bv, out)
    nc.compile()
    return nc


_DBG_N = [0]


def _dbg_dump(tc, name, src_ap, shape, dst=None):
    if not DEBUG:
        return
    nc = tc.nc
    _DBG_N[0] += 1
    key = f"{name}_{_DBG_N[0]}" if name else f"anon_{_DBG_N[0]}"
    if dst is None:
        dst = _DBG[name]
    pool = tc.tile_pool(name=f"dbgp_{key}", bufs=1)
    with pool as p:
        t = p.tile(shape, FP, tag=f"dbg_{key}")
        nc.vector.tensor_copy(t[:], src_ap)
        nc.sync.dma_start(dst, t[:])


def _body(tc, x, wq, bq, wk, wv, bv, out):
    nc = tc.nc

    with ExitStack() as ctx:
        const_pool = ctx.enter_context(tc.tile_pool(name="const", bufs=1))
        warm_in = const_pool.tile([P, P], BF, tag="warm_in")
        nc.vector.memset(warm_in[:], 1.0)
        ident_bf = const_pool.tile([P, P], BF, tag="ident_bf")
        make_identity(nc, ident_bf[:])
        ident_f = const_pool.tile([4, 4], FP, tag="ident_f")
        make_identity(nc, ident_f[:])
        ones_row = const_pool.tile([1, P], BF, tag="ones_row")
        nc.vector.memset(ones_row[:], 1.0)
        ones_col = const_pool.tile([P, 1], BF, tag="ones_col")
        nc.vector.memset(ones_col[:], 1.0)

        bias_pool = ctx.enter_context(tc.tile_pool(name="bias", bufs=1))
        big_pool = ctx.enter_context(tc.tile_pool(name="big", bufs=1))
        xT = [big_pool.tile([P, S], BF, tag=f"xT{d}", name=f"xT{d}")
              for d in range(D_CH)]
        qm = [big_pool.tile([P, S], BF, tag=f"qm{d}", name=f"qm{d}")
              for d in range(D_CH)]
        v = [big_pool.tile([P, H], BF, tag=f"v{t}", name=f"v{t}")
             for t in range(N_BF_K)]
        v8 = [big_pool.tile([P, 2, H], F8, tag=f"v8_{i}", name=f"v8_{i}")
              for i in range(N_F8_PAIRS)]
        gcol_all = bias_pool.tile([P, S_T], FP, tag="gcol_all")

        psum_mm = ctx.enter_context(tc.tile_pool(name="pmm", bufs=7,
                                                 space="PSUM"))
        psum_sm = ctx.enter_context(tc.tile_pool(name="psm", bufs=1,
                                                 space="PSUM"))

        xctx = ExitStack()
        wst_pool = xctx.enter_context(tc.tile_pool(name="wst", bufs=1))
        wbf_pool = xctx.enter_context(tc.tile_pool(name="wbf", bufs=1))
        m_pool = xctx.enter_context(tc.tile_pool(name="m", bufs=1))
        xs_pool = xctx.enter_context(tc.tile_pool(name="xs", bufs=1))
        xb_pool = xctx.enter_context(tc.tile_pool(name="xb", bufs=2))

        # ---- DMA issue order = consumption order: x0 first (unblocks
        # transposes), then Wq/Wk (M-make), remaining x, Wv (V-proj), biases.
        xs = [xs_pool.tile([P, 4, NB], FP, tag=f"xs{c}", name=f"xs{c}")
              for c in range(QB)]
        xb_of = {}

        def load_x(c, parts=2):
            xr = x[c * NB:(c + 1) * NB, :]
            n = P // parts
            for i in range(parts):
                nc.sync.dma_start(
                    xs[c][i * n:(i + 1) * n],
                    xr[i * n * 4:(i + 1) * n * 4].rearrange(
                        "(p a) d -> p a d", a=4))

        w_stg = {}

        def load_w(name, ap):
            for c in range(D_CH):
                t = wst_pool.tile([P, H], FP, tag=f"{name}s{c}",
                                  name=f"{name}s{c}")
                nc.sync.dma_start(t[0:64], ap[c * P:c * P + 64, :])
                nc.sync.dma_start(t[64:128], ap[c * P + 64:(c + 1) * P, :])
                w_stg[name, c] = t

        def load_w8(name, ap):
            # 4 consecutive rows per partition -> 8KB descriptors; the
            # strided transpose writes later restore natural d order.
            t = wst_pool.tile([P, 4, NB], FP, tag=f"{name}8s",
                              name=f"{name}8s")
            nc.sync.dma_start(
                t[0:64], ap[0:256].rearrange("(p a) h -> p a h", a=4))
            nc.sync.dma_start(
                t[64:128], ap[256:512].rearrange("(p a) h -> p a h", a=4))
            return t

        bq_stg = bias_pool.tile([1, H], FP, tag="bq_stg")
        nc.sync.dma_start(bq_stg[:], bq[None, :])
        bv_stg = bias_pool.tile([1, H], FP, tag="bv_stg")
        nc.sync.dma_start(bv_stg[:], bv[None, :])
        wq8_stg = load_w8("wq", wq)
        wk8_stg = load_w8("wk", wk)
        load_x(0, parts=4)
        for c in range(1, QB):
            load_x(c)
        load_w("wv", wv)

        # scalar ACT-table preload so the first real copy isn't delayed
        sc_warm = const_pool.tile([1, 4], FP, tag="sc_warm")
        nc.scalar.copy(sc_warm[:], ident_f[0:1, 0:4])

        # ---- warmup: ramp the PE clock while DMAs land ----
        warm_ps = psum_mm.tile([P, P], FP, tag="mm", name="warm_ps")
        with nc.named_scope("warmup"):
            for wi in range(80):
                nc.tensor.matmul(warm_ps[:], warm_in[:], warm_in[:],
                                 start=(wi == 0), stop=(wi == 79))

        def emit_transpose(c):
            with nc.named_scope(f"tp{c}"):
                xb = xb_pool.tile([P, 4, NB], BF, tag="xb")
                xb_of[c] = xb
                for a in range(4):
                    nc.scalar.copy(xb[:, a, :], xs[c][:, a, :])
                # d-outer so xT[d] chunks complete in consumption order
                for d in range(D_CH):
                    for a in range(4):
                        pt = psum_mm.tile([P, NB], FP, tag="mm")
                        pt_bf = pt[:].bitcast(BF)
                        nc.tensor.transpose(
                            pt_bf[:, 0:P], xb[:, a, d * P:(d + 1) * P],
                            ident_bf[:])
                        # sigma layout: col a*128+p holds seq row 4p+a
                        nc.vector.tensor_copy(
                            xT[d][:, c * NB + a * P:c * NB + (a + 1) * P],
                            pt_bf[:, 0:P])

        # ---- casts in DMA-arrival order on the DVE: biases, wq8, wk8 ----
        bq_row = bias_pool.tile([1, H], BF, tag="bq_row")
        nc.vector.tensor_copy(bq_row[:], bq_stg[:])
        bv_row = bias_pool.tile([1, H], BF, tag="bv_row")
        nc.vector.tensor_copy(bv_row[:], bv_stg[:])
        wq8 = wbf_pool.tile([P, 4, NB], BF, tag="wq8")
        nc.scalar.copy(wq8[:], wq8_stg[:])
        wk8 = wbf_pool.tile([P, 4, NB], BF, tag="wk8")
        nc.scalar.copy(wk8[:], wk8_stg[:])

        with nc.named_scope("bcast"):
            bv_ps = psum_mm.tile([P, NB], FP, tag="mm", name="bv_ps")
            nc.tensor.matmul(bv_ps[:], ones_row[:], bv_row[:], start=True,
                             stop=True)
            bv_full = bias_pool.tile([P, H], FP, tag="bv_full")
            nc.vector.tensor_copy(bv_full[:], bv_ps[:])

        # ---- M = Wq Wk^T (contract over h): transpose Wq/Wk, then MM ----
        # row d = 4p + a of the 8KB layout -> strided col write, like xT
        wT = {}
        with nc.named_scope("wtrans"):
            for name, src8 in (("wq", wq8), ("wk", wk8)):
                for ct in range(D_CH):  # h-chunk ct
                    t = m_pool.tile([P, H], BF, tag=f"{name}T{ct}",
                                    name=f"{name}T{ct}")
                    wT[name, ct] = t
                for ct in range(D_CH):
                    for a in range(4):
                        pt = psum_mm.tile([P, NB], FP, tag="mm")
                        pt_bf = pt[:].bitcast(BF)
                        nc.tensor.transpose(
                            pt_bf[:, 0:P],
                            src8[:, a, ct * P:(ct + 1) * P],
                            ident_bf[:])
                        dst = wT[name, ct][:, a:H:4]
                        if name == "wk":
                            nc.scalar.copy(dst, pt_bf[:, 0:P])
                        else:
                            nc.vector.tensor_copy(dst, pt_bf[:, 0:P])
        m_t = []
        with nc.named_scope("mmake"):
            for i in range(D_CH):
                ps = psum_mm.tile([P, NB], FP, tag="mm")
                for c in range(D_CH):
                    nc.tensor.matmul(
                        ps[:], wT["wq", c][:, i * P:(i + 1) * P],
                        wT["wk", c][:], start=(c == 0), stop=(c == D_CH - 1))
                mt = m_pool.tile([P, NB], BF, tag=f"m{i}", name=f"m{i}")
                nc.scalar.copy(mt[:], ps[:])
                m_t.append(mt)

        # ---- gamma prep: w_col[d] = (Wk bq) chunk columns, via wkT ----
        with nc.named_scope("gprep"):
            bq_col = []
            for c in range(D_CH):
                pt = psum_mm.tile([P, NB], FP, tag="mm")
                pt_bf = pt[:].bitcast(BF)
                nc.tensor.transpose(pt_bf[:, 0:1],
                                    bq_row[0:1, c * P:(c + 1) * P],
                                    ident_bf[0:1, 0:1])
                t = bias_pool.tile([P, 1], BF, tag=f"bqc{c}", name=f"bqc{c}")
                nc.vector.tensor_copy(t[:], pt_bf[:, 0:1])
                bq_col.append(t)
            wrp = psum_sm.tile([1, NB], FP, tag="sm", name="wrow")
            for c in range(D_CH):
                nc.tensor.matmul(wrp[:], bq_col[c][:], wT["wk", c][:],
                                 start=(c == 0), stop=(c == D_CH - 1))
            w_rowb = bias_pool.tile([1, NB], BF, tag="w_rowb")
            nc.scalar.copy(w_rowb[:], wrp[:])
            wf_ps = psum_mm.tile([P, NB], FP, tag="mm", name="wf_ps")
            nc.tensor.matmul(wf_ps[:], ones_row[:], w_rowb[:], start=True,
                             stop=True)
            w_full = bias_pool.tile([P, NB], BF, tag="w_full")
            nc.vector.tensor_copy(w_full[:], wf_ps[:])

        # ---- per chunk: qm-proj + gamma row; V-proj last (Wv lands last) --
        def emit_qm_g(c):
            ss = slice(c * NB, (c + 1) * NB)
            with nc.named_scope(f"qm{c}"):
                for j in range(D_CH):
                    ps = psum_mm.tile([P, NB], FP, tag="mm")
                    for i in range(D_CH):
                        nc.tensor.matmul(
                            ps[:], m_t[i][:, j * P:(j + 1) * P], xT[i][:, ss],
                            start=(i == 0), stop=(i == D_CH - 1))
                    nc.scalar.copy(qm[j][:, ss], ps[:])
            with nc.named_scope(f"g{c}"):
                # gamma[kt][p] = x[seq 4p+a] . w  (DVE mul + free-reduce;
                # partition p of xb[:, a, :] is exactly k-tile kt part. p)
                xb = xb_of[c]
                for a in range(4):
                    kt = c * 4 + a
                    gt = xb_pool.tile([P, NB], FP, tag="gtmp")
                    nc.gpsimd.tensor_mul(gt[:], xb[:, a, :], w_full[:])
                    rt = xb_pool.tile([P, 1], FP, tag="grt")
                    nc.vector.tensor_reduce(rt[:], gt[:],
                                            axis=mybir.AxisListType.X,
                                            op=mybir.AluOpType.add)
                    nc.vector.tensor_scalar(gcol_all[:, kt:kt + 1], rt[:],
                                            SCALE, -C_SHIFT,
                                            mybir.AluOpType.mult,
                                            mybir.AluOpType.add)

        emit_transpose(0)
        emit_qm_g(0)
        for c in range(1, QB):
            emit_transpose(c)
            emit_qm_g(c)

        # wv casts here: the DVE reaches them only after the early casts
        w_bf = {}
        for c in range(D_CH):
            t = wbf_pool.tile([P, H], BF, tag=f"wvb{c}", name=f"wvb{c}")
            nc.vector.tensor_copy(t[:], w_stg["wv", c][:])
            w_bf["wv", c] = t

        for c in range(QB):
            with nc.named_scope(f"v{c}"):
                for st in range(c * 4, c * 4 + 4):
                    ts = slice(st * P, (st + 1) * P)
                    ps = psum_mm.tile([P, NB], FP, tag="mm")
                    for d in range(D_CH):
                        nc.tensor.matmul(ps[:], xT[d][:, ts],
                                         w_bf["wv", d][:],
                                         start=(d == 0), stop=(d == D_CH - 1))
                    if st < N_BF_K:
                        nc.vector.tensor_add(v[st][:], ps[:], bv_full[:])
                    else:
                        pi, half = (st - N_BF_K) // 2, (st - N_BF_K) % 2
                        vtmp = xb_pool.tile([P, H], BF, tag="vtmp")
                        nc.vector.tensor_add(vtmp[:], ps[:], bv_full[:])
                        nc.vector.tensor_copy(v8[pi][:, half, :], vtmp[:])

        if DEBUG:
            nc.sync.dma_start(_DBG["dbg_gcol"], gcol_all[:])
        _dbg_dump(tc, "dbg_xt0", xT[0][:], [P, S])
        _dbg_dump(tc, "dbg_qm0", qm[0][:], [P, S])
        if DEBUG:
            xt_r = _DBG["dbg_xt"].rearrange("p (c s) -> p c s", c=4)
            qm_r = _DBG["dbg_qm"].rearrange("p (c s) -> p c s", c=4)
            for d in range(D_CH):
                _dbg_dump(tc, "xtd", xT[d][:], [P, S], dst=xt_r[:, d])
                _dbg_dump(tc, "qmd", qm[d][:], [P, S], dst=qm_r[:, d])
        _dbg_dump(tc, "dbg_m0", m_t[0][:], [P, NB])
        _dbg_dump(tc, "dbg_v0", v[0][:], [P, NB])
        _dbg_dump(tc, "dbg_v80", v8[0][:, 0, :], [P, NB])
        xctx.close()

        # ---- attention, software-pipelined across query blocks ----
        p_pool = ctx.enter_context(tc.tile_pool(name="pp", bufs=2))
        sm_pool = ctx.enter_context(tc.tile_pool(name="sm", bufs=2))
        o_pool = ctx.enter_context(tc.tile_pool(name="o", bufs=2))

        p_t = {}    # (qb, k) -> bf16 tile
        p8_t = {}   # (qb, pair) -> fp8 paired tile
        s_sum = {}  # qb -> fp32 denominator sum tile
        rcols = {}  # qb -> list of 4 [128,1] reciprocal cols (a-order)

        def emit_scores(qb):
            qs = slice(qb * NB, (qb + 1) * NB)
            with nc.named_scope(f"sc{qb}"):
                for k in range(S_T):
                    ks = slice(k * P, (k + 1) * P)
                    ps = psum_mm.tile([P, NB], FP, tag="mm")
                    for d in range(D_CH):
                        nc.tensor.matmul(ps[:], xT[d][:, ks], qm[d][:, qs],
                                         start=(d == 0), stop=(d == D_CH - 1))
                    if k < N_BF_K:
                        t = p_pool.tile([P, NB], BF, tag=f"p{k}",
                                        name=f"p{qb}_{k}")
                        p_t[qb, k] = t
                        nc.scalar.activation(t[:], ps[:], EXP, scale=SCALE,
                                             bias=gcol_all[:, k:k + 1])
                    else:
                        pi, half = (k - N_BF_K) // 2, (k - N_BF_K) % 2
                        if half == 0:
                            p8_t[qb, pi] = p_pool.tile(
                                [P, 2, NB], F8, tag=f"p8_{pi}",
                                name=f"p8_{qb}_{pi}")
                        nc.scalar.activation(p8_t[qb, pi][:, half, :], ps[:],
                                             EXP, scale=SCALE,
                                             bias=gcol_all[:, k:k + 1])
                # fp32 denominator tree over all 16 P tile views
                aps = [p_t[qb, k][:] for k in range(N_BF_K)]
                for pi in range(N_F8_PAIRS):
                    aps += [p8_t[qb, pi][:, 0, :], p8_t[qb, pi][:, 1, :]]
                gs = []
                for g in range(4):
                    t = sm_pool.tile([P, NB], FP, tag=f"gs{g}",
                                     name=f"gs{qb}_{g}")
                    nc.vector.tensor_add(t[:], aps[4 * g], aps[4 * g + 1])
                    nc.vector.tensor_add(t[:], t[:], aps[4 * g + 2])
                    nc.vector.tensor_add(t[:], t[:], aps[4 * g + 3])
                    gs.append(t)
                nc.vector.tensor_add(gs[0][:], gs[0][:], gs[1][:])
                nc.vector.tensor_add(gs[2][:], gs[2][:], gs[3][:])
                nc.vector.tensor_add(gs[0][:], gs[0][:], gs[2][:])
                sb = sm_pool.tile([P, NB], BF, tag="sbf", name=f"sbf{qb}")
                nc.vector.tensor_copy(sb[:], gs[0][:])
                s_sum[qb] = sb

        def emit_denom(qb):
            with nc.named_scope(f"dn{qb}"):
                dps = psum_sm.tile([1, NB], FP, tag="sm", name=f"dps{qb}")
                nc.tensor.matmul(dps[:], ones_col[:], s_sum[qb][:],
                                 start=True, stop=True)
                drow = sm_pool.tile([1, NB], FP, tag="drow", name=f"dr{qb}")
                nc.scalar.copy(drow[:], dps[:])
                if DEBUG and qb == 0:
                    nc.sync.dma_start(_DBG["dbg_drow0"], drow[:])
                rc = sm_pool.tile([P, 4], FP, tag="rc", name=f"rc{qb}")
                for a in range(4):
                    rp = psum_sm.tile([P, 1], FP, tag="sm",
                                      name=f"rtp{qb}_{a}")
                    nc.tensor.transpose(rp[:, 0:1],
                                        drow[0:1, a * P:(a + 1) * P],
                                        ident_f[0:1, 0:1])
                    nc.vector.reciprocal(rc[:, a:a + 1], rp[:, 0:1])
                rcols[qb] = rc

        def emit_av(qb):
            with nc.named_scope(f"av{qb}"):
                o_blk = o_pool.tile([P, 4, NB], FP, tag="ob", name=f"ob{qb}")
                for a in range(4):
                    asl = slice(a * P, (a + 1) * P)
                    ps = psum_mm.tile([P, NB], FP, tag="mm")
                    for k in range(N_BF_K):
                        nc.tensor.matmul(ps[:], p_t[qb, k][:, asl],
                                         v[k][:], start=(k == 0), stop=False)
                    for pi in range(N_F8_PAIRS):
                        nc.tensor.matmul(ps[:], p8_t[qb, pi][:, :, asl],
                                         v8[pi][:], start=False,
                                         stop=(pi == N_F8_PAIRS - 1),
                                         perf_mode=DR)
                    nc.vector.tensor_scalar_mul(o_blk[:, a, :], ps[:],
                                                rcols[qb][:, a:a + 1])
                for r in range(4):
                    q0 = qb * NB + r * P
                    nc.sync.dma_start(
                        out[q0:q0 + P, :].rearrange("(p a) h -> p a h", a=4),
                        o_blk[32 * r:32 * (r + 1)])

        for qb in range(QB):
            emit_scores(qb)
            if qb == 0:
                _dbg_dump(tc, "dbg_p00", p_t[0, 0][:], [P, NB])
                _dbg_dump(tc, "dbg_p800", p8_t[0, 0][:, 0, :], [P, NB])
                _dbg_dump(tc, "dbg_ssum0", s_sum[0][:], [P, NB])
                if DEBUG:
                    pall = _DBG["dbg_pall"].rearrange("p (k n) -> p k n",
                                                      k=S_T)
                    for k in range(N_BF_K):
                        _dbg_dump(tc, "pk", p_t[0, k][:], [P, NB],
                                  dst=pall[:, k])
                    for pi in range(N_F8_PAIRS):
                        for hf in range(2):
                            _dbg_dump(tc, "p8k", p8_t[0, pi][:, hf, :],
                                      [P, NB],
                                      dst=pall[:, N_BF_K + 2 * pi + hf])
            if qb > 0:
                emit_denom(qb - 1)
                if qb == 1:
                    _dbg_dump(tc, "dbg_rc00", rcols[0][:, 0:1], [P, 1])
                emit_av(qb - 1)
        emit_denom(QB - 1)
        emit_av(QB - 1)


_NC = None


def kernel(**inputs):
    global _NC
    if _NC is None:
        _NC = _build()
    x = np.ascontiguousarray(np.asarray(inputs["x"], dtype=np.float32))
    shared = {
        k: np.ascontiguousarray(np.asarray(inputs[k], dtype=np.float32))
        for k in ("Wq", "bq", "Wk", "Wv", "bv")
    }
    in_maps = [dict(shared, x=np.ascontiguousarray(x[b])) for b in range(B)]
    res = run_bass_kernel_spmd(_NC, in_maps, core_ids=list(range(B)))
    return np.stack([res.results[b]["out"] for b in range(B)], axis=0)


# revision 68
# speedup vs baseline: 1.0359x; 1.0359x over previous
"""Distributed attention kernel for Trainium2 (8 NeuronCores).

Problem: nn_Attention (B=8, S=2048, d_model=512, d_hid=512, fp32).
Sharding: data-parallel over batch - one batch element per core, no
collectives. Each core computes softmax(Q K^T / sqrt(d)) V for its
[2048, 512] slice.

Per-core plan (matmul operands bf16 except where noted):
  * M-trick: scores = (x Wq)(x Wk)^T = x (Wq Wk^T) x^T. Precompute
    M = Wq Wk^T (8k PE cycles) and skip the K projection entirely
    (-32k cycles). qm = M^T x^T plays the role of Q^T; x^T itself
    plays the role of K^T. Bias handling stays exact: the Q-side bias
    terms are constant along the softmax axis (drop), the K-side term
    gamma[k] = x[k] . (Wk bq) is folded into the exp bias.
  * fp8 e4m3 + DoubleRow on half the AV k-chunks: 2x PE throughput on
    those chunks (-33k cycles); denominator computed from the
    quantized P so the correlated part of the quantization error
    cancels. Predicted rel-l2 ~1.75e-2 < 2e-2 gate.
  * Denominator in fp32: DVE tree-sum of the 16 P tiles, then one
    fp32 ones-matmul -> [1, 512]; transposes + DVE reciprocal give
    1/denom per-partition columns.
  * Software pipelining: AV(qb-1) is emitted after scores(qb), so the
    denom chain of a block hides under the next block's scores.
  * DMA with 8KB descriptors: x is loaded 4-consecutive-rows-per-
    partition; the output is assembled likewise by taking stride-4
    stationary slices of P in the AV matmul (o_blk[p, a, :] = row
    4p+a), then written with 4 partition-range dma_starts per block.
  * No max-subtraction in softmax: scores ~ N(0,1).
"""

import sys

for _p in ("/opt/trn_rl_repo",):
    if _p not in sys.path:
        sys.path.append(_p)

from contextlib import ExitStack

import numpy as np

import concourse.bass as bass
import concourse.mybir as mybir
import concourse.tile as tile
from concourse import bacc
from concourse.bass_utils import run_bass_kernel_spmd
from concourse.masks import make_identity

B = 8
S = 2048
D = 512
H = 512
P = 128
NB = 512  # matmul free-dim / PSUM bank (fp32)
FP = mybir.dt.float32
BF = mybir.dt.bfloat16
F8 = mybir.dt.float8e4
DR = mybir.MatmulPerfMode.DoubleRow
SCALE = 1.0 / float(np.sqrt(H))

D_CH = D // P   # 4 contraction chunks
S_T = S // P    # 16 sequence tiles
QB = S // NB    # 4 query blocks
N_BF_K = 4      # k-tiles 0..3 computed in bf16
N_F8_PAIRS = 6  # k-tiles 4..15 as 6 fp8 DoubleRow pairs
EXP = mybir.ActivationFunctionType.Exp
IDENT = mybir.ActivationFunctionType.Identity
# Constant subtracted from logits before exp (softmax-invariant): keeps
# exp(s) well under the e4m3 inf threshold (~240) for s up to ~7.5 sigma.
C_SHIFT = 2.0


DEBUG = False
_DBG = {}


def _build():
    nc = bacc.Bacc("TRN2", target_bir_lowering=False, debug=False)
    x = nc.dram_tensor("x", [S, D], FP, kind="ExternalInput").ap()
    wq = nc.dram_tensor("Wq", [D, H], FP, kind="ExternalInput").ap()
    bq = nc.dram_tensor("bq", [H], FP, kind="ExternalInput").ap()
    wk = nc.dram_tensor("Wk", [D, H], FP, kind="ExternalInput").ap()
    wv = nc.dram_tensor("Wv", [D, H], FP, kind="ExternalInput").ap()
    bv = nc.dram_tensor("bv", [H], FP, kind="ExternalInput").ap()
    out = nc.dram_tensor("out", [S, H], FP, kind="ExternalOutput").ap()
    if DEBUG:
        for nm, shp in (("dbg_xt0", [P, S]), ("dbg_qm0", [P, S]),
                        ("dbg_m0", [P, NB]), ("dbg_p00", [P, NB]),
                        ("dbg_p800", [P, NB]), ("dbg_rc00", [P, 1]),
                        ("dbg_gcol", [P, S_T]), ("dbg_ssum0", [P, NB]),
                        ("dbg_drow0", [1, NB]), ("dbg_drow0v", [1, NB]),
                        ("dbg_xt", [P, 4 * S]), ("dbg_qm", [P, 4 * S]),
                        ("dbg_pall", [P, S_T * NB]),
                        ("dbg_v0", [P, NB]), ("dbg_v80", [P, NB])):
            _DBG[nm] = nc.dram_tensor(nm, shp, FP, kind="ExternalOutput").ap()

    with tile.TileContext(nc, pool_alloc_mode="queue") as tc:
        _body(tc, x, wq, bq, wk, wv, bv, out)
    nc.compile()
    return nc


_DBG_N = [0]


def _dbg_dump(tc, name, src_ap, shape, dst=None):
    if not DEBUG:
        return
    nc = tc.nc
    _DBG_N[0] += 1
    key = f"{name}_{_DBG_N[0]}" if name else f"anon_{_DBG_N[0]}"
    if dst is None:
        dst = _DBG[name]
    pool = tc.tile_pool(name=f"dbgp_{key}", bufs=1)
    with pool as p:
        t = p.tile(shape, FP, tag=f"dbg_{key}")
        nc.vector.tensor_copy(t[:], src_ap)
        nc.sync.dma_start(dst, t[:])


def _body(tc, x, wq, bq, wk, wv, bv, out):
    nc = tc.nc

    with ExitStack() as ctx:
        const_pool = ctx.enter_context(tc.tile_pool(name="const", bufs=1))
        warm_in = const_pool.tile([P, P], BF, tag="warm_in")
        nc.vector.memset(warm_in[:], 1.0)
        ident_bf = const_pool.tile([P, P], BF, tag="ident_bf")
        make_identity(nc, ident_bf[:])
        ident_f = const_pool.tile([4, 4], FP, tag="ident_f")
        make_identity(nc, ident_f[:])
        ones_row = const_pool.tile([1, P], BF, tag="ones_row")
        nc.vector.memset(ones_row[:], 1.0)
        ones_col = const_pool.tile([P, 1], BF, tag="ones_col")
        nc.vector.memset(ones_col[:], 1.0)

        bias_pool = ctx.enter_context(tc.tile_pool(name="bias", bufs=1))
        big_pool = ctx.enter_context(tc.tile_pool(name="big", bufs=1))
        xT = [big_pool.tile([P, S], BF, tag=f"xT{d}", name=f"xT{d}")
              for d in range(D_CH)]
        qm = [big_pool.tile([P, S], BF, tag=f"qm{d}", name=f"qm{d}")
              for d in range(D_CH)]
        v = [big_pool.tile([P, H], BF, tag=f"v{t}", name=f"v{t}")
             for t in range(N_BF_K)]
        v8 = [big_pool.tile([P, 2, H], F8, tag=f"v8_{i}", name=f"v8_{i}")
              for i in range(N_F8_PAIRS)]
        gcol_all = bias_pool.tile([P, S_T], FP, tag="gcol_all")

        psum_mm = ctx.enter_context(tc.tile_pool(name="pmm", bufs=7,
                                                 space="PSUM"))
        psum_sm = ctx.enter_context(tc.tile_pool(name="psm", bufs=1,
                                                 space="PSUM"))

        xctx = ExitStack()
        wst_pool = xctx.enter_context(tc.tile_pool(name="wst", bufs=1))
        wbf_pool = xctx.enter_context(tc.tile_pool(name="wbf", bufs=1))
        m_pool = xctx.enter_context(tc.tile_pool(name="m", bufs=1))
        xs_pool = xctx.enter_context(tc.tile_pool(name="xs", bufs=1))
        xb_pool = xctx.enter_context(tc.tile_pool(name="xb", bufs=2))

        # ---- DMA issue order = consumption order: x0 first (unblocks
        # transposes), then Wq/Wk (M-make), remaining x, Wv (V-proj), biases.
        xs = [xs_pool.tile([P, 4, NB], FP, tag=f"xs{c}", name=f"xs{c}")
              for c in range(QB)]
        xb_of = {}

        def load_x(c, parts=2):
            xr = x[c * NB:(c + 1) * NB, :]
            n = P // parts
            for i in range(parts):
                nc.sync.dma_start(
                    xs[c][i * n:(i + 1) * n],
                    xr[i * n * 4:(i + 1) * n * 4].rearrange(
                        "(p a) d -> p a d", a=4))

        w_stg = {}

        def load_w(name, ap):
            for c in range(D_CH):
                t = wst_pool.tile([P, H], FP, tag=f"{name}s{c}",
                                  name=f"{name}s{c}")
                nc.sync.dma_start(t[0:64], ap[c * P:c * P + 64, :])
                nc.sync.dma_start(t[64:128], ap[c * P + 64:(c + 1) * P, :])
                w_stg[name, c] = t

        def load_w8(name, ap):
            # 4 consecutive rows per partition -> 8KB descriptors; the
            # strided transpose writes later restore natural d order.
            t = wst_pool.tile([P, 4, NB], FP, tag=f"{name}8s",
                              name=f"{name}8s")
            nc.sync.dma_start(
                t[0:64], ap[0:256].rearrange("(p a) h -> p a h", a=4))
            nc.sync.dma_start(
                t[64:128], ap[256:512].rearrange("(p a) h -> p a h", a=4))
            return t

        bq_stg = bias_pool.tile([1, H], FP, tag="bq_stg")
        nc.sync.dma_start(bq_stg[:], bq[None, :])
        bv_stg = bias_pool.tile([1, H], FP, tag="bv_stg")
        nc.sync.dma_start(bv_stg[:], bv[None, :])
        load_x(0, parts=4)
        wq8_stg = load_w8("wq", wq)
        wk8_stg = load_w8("wk", wk)
        for c in range(1, QB):
            load_x(c)
        load_w("wv", wv)

        # scalar ACT-table preload so the first real copy isn't delayed
        sc_warm = const_pool.tile([1, 4], FP, tag="sc_warm")
        nc.scalar.copy(sc_warm[:], ident_f[0:1, 0:4])

        # ---- warmup: ramp the PE clock while DMAs land ----
        warm_ps = psum_mm.tile([P, P], FP, tag="mm", name="warm_ps")
        with nc.named_scope("warmup"):
            for wi in range(80):
                nc.tensor.matmul(warm_ps[:], warm_in[:], warm_in[:],
                                 start=(wi == 0), stop=(wi == 79))

        def emit_transpose(c):
            with nc.named_scope(f"tp{c}"):
                xb = xb_pool.tile([P, 4, NB], BF, tag="xb")
                xb_of[c] = xb
                for a in range(4):
                    nc.scalar.copy(xb[:, a, :], xs[c][:, a, :])
                # d-outer so xT[d] chunks complete in consumption order
                for d in range(D_CH):
                    for a in range(4):
                        pt = psum_mm.tile([P, NB], FP, tag="mm")
                        pt_bf = pt[:].bitcast(BF)
                        nc.tensor.transpose(
                            pt_bf[:, 0:P], xb[:, a, d * P:(d + 1) * P],
                            ident_bf[:])
                        # sigma layout: col a*128+p holds seq row 4p+a
                        nc.vector.tensor_copy(
                            xT[d][:, c * NB + a * P:c * NB + (a + 1) * P],
                            pt_bf[:, 0:P])

        # ---- casts in DMA-arrival order on the DVE: biases, wq8, wk8 ----
        bq_row = bias_pool.tile([1, H], BF, tag="bq_row")
        nc.vector.tensor_copy(bq_row[:], bq_stg[:])
        bv_row = bias_pool.tile([1, H], BF, tag="bv_row")
        nc.vector.tensor_copy(bv_row[:], bv_stg[:])
        wq8 = wbf_pool.tile([P, 4, NB], BF, tag="wq8")
        nc.scalar.copy(wq8[:], wq8_stg[:])
        wk8 = wbf_pool.tile([P, 4, NB], BF, tag="wk8")
        nc.scalar.copy(wk8[:], wk8_stg[:])

        # x0 transposes: x0 lands before the weights; keep the PE busy
        emit_transpose(0)

        with nc.named_scope("bcast"):
            bv_ps = psum_mm.tile([P, NB], FP, tag="mm", name="bv_ps")
            nc.tensor.matmul(bv_ps[:], ones_row[:], bv_row[:], start=True,
                             stop=True)
            bv_full = bias_pool.tile([P, H], FP, tag="bv_full")
            nc.vector.tensor_copy(bv_full[:], bv_ps[:])

        # ---- M = Wq Wk^T (contract over h): transpose Wq/Wk, then MM ----
        # row d = 4p + a of the 8KB layout -> strided col write, like xT
        wT = {}
        with nc.named_scope("wtrans"):
            for name, src8 in (("wq", wq8), ("wk", wk8)):
                for ct in range(D_CH):  # h-chunk ct
                    t = m_pool.tile([P, H], BF, tag=f"{name}T{ct}",
                                    name=f"{name}T{ct}")
                    wT[name, ct] = t
                for ct in range(D_CH):
                    for a in range(4):
                        pt = psum_mm.tile([P, NB], FP, tag="mm")
                        pt_bf = pt[:].bitcast(BF)
                        nc.tensor.transpose(
                            pt_bf[:, 0:P],
                            src8[:, a, ct * P:(ct + 1) * P],
                            ident_bf[:])
                        dst = wT[name, ct][:, a:H:4]
                        if name == "wk":
                            nc.scalar.copy(dst, pt_bf[:, 0:P])
                        else:
                            nc.vector.tensor_copy(dst, pt_bf[:, 0:P])
        m_t = []
        with nc.named_scope("mmake"):
            for i in range(D_CH):
                ps = psum_mm.tile([P, NB], FP, tag="mm")
                for c in range(D_CH):
                    nc.tensor.matmul(
                        ps[:], wT["wq", c][:, i * P:(i + 1) * P],
                        wT["wk", c][:], start=(c == 0), stop=(c == D_CH - 1))
                mt = m_pool.tile([P, NB], BF, tag=f"m{i}", name=f"m{i}")
                nc.scalar.copy(mt[:], ps[:])
                m_t.append(mt)

        # ---- gamma prep: w_col[d] = (Wk bq) chunk columns, via wkT ----
        with nc.named_scope("gprep"):
            bq_col = []
            for c in range(D_CH):
                pt = psum_mm.tile([P, NB], FP, tag="mm")
                pt_bf = pt[:].bitcast(BF)
                nc.tensor.transpose(pt_bf[:, 0:1],
                                    bq_row[0:1, c * P:(c + 1) * P],
                                    ident_bf[0:1, 0:1])
                t = bias_pool.tile([P, 1], BF, tag=f"bqc{c}", name=f"bqc{c}")
                nc.vector.tensor_copy(t[:], pt_bf[:, 0:1])
                bq_col.append(t)
            wrp = psum_sm.tile([1, NB], FP, tag="sm", name="wrow")
            for c in range(D_CH):
                nc.tensor.matmul(wrp[:], bq_col[c][:], wT["wk", c][:],
                                 start=(c == 0), stop=(c == D_CH - 1))
            w_rowb = bias_pool.tile([1, NB], BF, tag="w_rowb")
            nc.scalar.copy(w_rowb[:], wrp[:])
            wf_ps = psum_mm.tile([P, NB], FP, tag="mm", name="wf_ps")
            nc.tensor.matmul(wf_ps[:], ones_row[:], w_rowb[:], start=True,
                             stop=True)
            w_full = bias_pool.tile([P, NB], BF, tag="w_full")
            nc.vector.tensor_copy(w_full[:], wf_ps[:])

        # ---- per chunk: qm-proj + gamma row; V-proj last (Wv lands last) --
        def emit_qm_g(c):
            ss = slice(c * NB, (c + 1) * NB)
            with nc.named_scope(f"qm{c}"):
                for j in range(D_CH):
                    ps = psum_mm.tile([P, NB], FP, tag="mm")
                    for i in range(D_CH):
                        nc.tensor.matmul(
                            ps[:], m_t[i][:, j * P:(j + 1) * P], xT[i][:, ss],
                            start=(i == 0), stop=(i == D_CH - 1))
                    nc.scalar.copy(qm[j][:, ss], ps[:])
            with nc.named_scope(f"g{c}"):
                # gamma[kt][p] = x[seq 4p+a] . w  (DVE mul + free-reduce;
                # partition p of xb[:, a, :] is exactly k-tile kt part. p)
                xb = xb_of[c]
                for a in range(4):
                    kt = c * 4 + a
                    gt = xb_pool.tile([P, NB], FP, tag="gtmp")
                    nc.gpsimd.tensor_mul(gt[:], xb[:, a, :], w_full[:])
                    rt = xb_pool.tile([P, 1], FP, tag="grt")
                    nc.vector.tensor_reduce(rt[:], gt[:],
                                            axis=mybir.AxisListType.X,
                                            op=mybir.AluOpType.add)
                    nc.vector.tensor_scalar(gcol_all[:, kt:kt + 1], rt[:],
                                            SCALE, -C_SHIFT,
                                            mybir.AluOpType.mult,
                                            mybir.AluOpType.add)

        emit_qm_g(0)
        for c in range(1, QB):
            emit_transpose(c)
            emit_qm_g(c)

        # wv casts here: the DVE reaches them only after the early casts
        w_bf = {}
        for c in range(D_CH):
            t = wbf_pool.tile([P, H], BF, tag=f"wvb{c}", name=f"wvb{c}")
            nc.vector.tensor_copy(t[:], w_stg["wv", c][:])
            w_bf["wv", c] = t

        for c in range(QB):
            with nc.named_scope(f"v{c}"):
                for st in range(c * 4, c * 4 + 4):
                    ts = slice(st * P, (st + 1) * P)
                    ps = psum_mm.tile([P, NB], FP, tag="mm")
                    for d in range(D_CH):
                        nc.tensor.matmul(ps[:], xT[d][:, ts],
                                         w_bf["wv", d][:],
                                         start=(d == 0), stop=(d == D_CH - 1))
                    if st < N_BF_K:
                        nc.vector.tensor_add(v[st][:], ps[:], bv_full[:])
                    else:
                        pi, half = (st - N_BF_K) // 2, (st - N_BF_K) % 2
                        vtmp = xb_pool.tile([P, H], BF, tag="vtmp")
                        nc.vector.tensor_add(vtmp[:], ps[:], bv_full[:])
                        nc.vector.tensor_copy(v8[pi][:, half, :], vtmp[:])

        if DEBUG:
            nc.sync.dma_start(_DBG["dbg_gcol"], gcol_all[:])
        _dbg_dump(tc, "dbg_xt0", xT[0][:], [P, S])
        _dbg_dump(tc, "dbg_qm0", qm[0][:], [P, S])
        if DEBUG:
            xt_r = _DBG["dbg_xt"].rearrange("p (c s) -> p c s", c=4)
            qm_r = _DBG["dbg_qm"].rearrange("p (c s) -> p c s", c=4)
            for d in range(D_CH):
                _dbg_dump(tc, "xtd", xT[d][:], [P, S], dst=xt_r[:, d])
                _dbg_dump(tc, "qmd", qm[d][:], [P, S], dst=qm_r[:, d])
        _dbg_dump(tc, "dbg_m0", m_t[0][:], [P, NB])
        _dbg_dump(tc, "dbg_v0", v[0][:], [P, NB])
        _dbg_dump(tc, "dbg_v80", v8[0][:, 0, :], [P, NB])
        xctx.close()

        # ---- attention, software-pipelined across query blocks ----
        p_pool = ctx.enter_context(tc.tile_pool(name="pp", bufs=2))
        sm_pool = ctx.enter_context(tc.tile_pool(name="sm", bufs=2))
        o_pool = ctx.enter_context(tc.tile_pool(name="o", bufs=2))

        p_t = {}    # (qb, k) -> bf16 tile
        p8_t = {}   # (qb, pair) -> fp8 paired tile
        s_sum = {}  # qb -> fp32 denominator sum tile
        rcols = {}  # qb -> list of 4 [128,1] reciprocal cols (a-order)

        def emit_scores(qb):
            qs = slice(qb * NB, (qb + 1) * NB)
            with nc.named_scope(f"sc{qb}"):
                for k in range(S_T):
                    ks = slice(k * P, (k + 1) * P)
                    ps = psum_mm.tile([P, NB], FP, tag="mm")
                    for d in range(D_CH):
                        nc.tensor.matmul(ps[:], xT[d][:, ks], qm[d][:, qs],
                                         start=(d == 0), stop=(d == D_CH - 1))
                    if k < N_BF_K:
                        t = p_pool.tile([P, NB], BF, tag=f"p{k}",
                                        name=f"p{qb}_{k}")
                        p_t[qb, k] = t
                        nc.scalar.activation(t[:], ps[:], EXP, scale=SCALE,
                                             bias=gcol_all[:, k:k + 1])
                    else:
                        pi, half = (k - N_BF_K) // 2, (k - N_BF_K) % 2
                        if half == 0:
                            p8_t[qb, pi] = p_pool.tile(
                                [P, 2, NB], F8, tag=f"p8_{pi}",
                                name=f"p8_{qb}_{pi}")
                        nc.scalar.activation(p8_t[qb, pi][:, half, :], ps[:],
                                             EXP, scale=SCALE,
                                             bias=gcol_all[:, k:k + 1])
                # fp32 denominator tree over all 16 P tile views
                aps = [p_t[qb, k][:] for k in range(N_BF_K)]
                for pi in range(N_F8_PAIRS):
                    aps += [p8_t[qb, pi][:, 0, :], p8_t[qb, pi][:, 1, :]]
                gs = []
                for g in range(4):
                    t = sm_pool.tile([P, NB], FP, tag=f"gs{g}",
                                     name=f"gs{qb}_{g}")
                    nc.vector.tensor_add(t[:], aps[4 * g], aps[4 * g + 1])
                    nc.vector.tensor_add(t[:], t[:], aps[4 * g + 2])
                    nc.vector.tensor_add(t[:], t[:], aps[4 * g + 3])
                    gs.append(t)
                nc.vector.tensor_add(gs[0][:], gs[0][:], gs[1][:])
                nc.vector.tensor_add(gs[2][:], gs[2][:], gs[3][:])
                nc.vector.tensor_add(gs[0][:], gs[0][:], gs[2][:])
                sb = sm_pool.tile([P, NB], BF, tag="sbf", name=f"sbf{qb}")
                nc.vector.tensor_copy(sb[:], gs[0][:])
                s_sum[qb] = sb

        def emit_denom(qb):
            with nc.named_scope(f"dn{qb}"):
                dps = psum_sm.tile([1, NB], FP, tag="sm", name=f"dps{qb}")
                nc.tensor.matmul(dps[:], ones_col[:], s_sum[qb][:],
                                 start=True, stop=True)
                drow = sm_pool.tile([1, NB], FP, tag="drow", name=f"dr{qb}")
                nc.scalar.copy(drow[:], dps[:])
                if DEBUG and qb == 0:
                    nc.sync.dma_start(_DBG["dbg_drow0"], drow[:])
                rc = sm_pool.tile([P, 4], FP, tag="rc", name=f"rc{qb}")
                for a in range(4):
                    rp = psum_sm.tile([P, 1], FP, tag="sm",
                                      name=f"rtp{qb}_{a}")
                    nc.tensor.transpose(rp[:, 0:1],
                                        drow[0:1, a * P:(a + 1) * P],
                                        ident_f[0:1, 0:1])
                    nc.vector.reciprocal(rc[:, a:a + 1], rp[:, 0:1])
                rcols[qb] = rc

        def emit_av(qb):
            with nc.named_scope(f"av{qb}"):
                o_blk = o_pool.tile([P, 4, NB], FP, tag="ob", name=f"ob{qb}")
                for a in range(4):
                    asl = slice(a * P, (a + 1) * P)
                    ps = psum_mm.tile([P, NB], FP, tag="mm")
                    for k in range(N_BF_K):
                        nc.tensor.matmul(ps[:], p_t[qb, k][:, asl],
                                         v[k][:], start=(k == 0), stop=False)
                    for pi in range(N_F8_PAIRS):
                        nc.tensor.matmul(ps[:], p8_t[qb, pi][:, :, asl],
                                         v8[pi][:], start=False,
                                         stop=(pi == N_F8_PAIRS - 1),
                                         perf_mode=DR)
                    nc.vector.tensor_scalar_mul(o_blk[:, a, :], ps[:],
                                                rcols[qb][:, a:a + 1])
                for r in range(4):
                    q0 = qb * NB + r * P
                    nc.sync.dma_start(
                        out[q0:q0 + P, :].rearrange("(p a) h -> p a h", a=4),
                        o_blk[32 * r:32 * (r + 1)])

        for qb in range(QB):
            emit_scores(qb)
            if qb == 0:
                _dbg_dump(tc, "dbg_p00", p_t[0, 0][:], [P, NB])
                _dbg_dump(tc, "dbg_p800", p8_t[0, 0][:, 0, :], [P, NB])
                _dbg_dump(tc, "dbg_ssum0", s_sum[0][:], [P, NB])
                if DEBUG:
                    pall = _DBG["dbg_pall"].rearrange("p (k n) -> p k n",
                                                      k=S_T)
                    for k in range(N_BF_K):
                        _dbg_dump(tc, "pk", p_t[0, k][:], [P, NB],
                                  dst=pall[:, k])
                    for pi in range(N_F8_PAIRS):
                        for hf in range(2):
                            _dbg_dump(tc, "p8k", p8_t[0, pi][:, hf, :],
                                      [P, NB],
                                      dst=pall[:, N_BF_K + 2 * pi + hf])
            if qb > 0:
                emit_denom(qb - 1)
                if qb == 1:
                    _dbg_dump(tc, "dbg_rc00", rcols[0][:, 0:1], [P, 1])
                emit_av(qb - 1)
        emit_denom(QB - 1)
        emit_av(QB - 1)


_NC = None


def kernel(**inputs):
    global _NC
    if _NC is None:
        _NC = _build()
    x = np.ascontiguousarray(np.asarray(inputs["x"], dtype=np.float32))
    shared = {
        k: np.ascontiguousarray(np.asarray(inputs[k], dtype=np.float32))
        for k in ("Wq", "bq", "Wk", "Wv", "bv")
    }
    in_maps = [dict(shared, x=np.ascontiguousarray(x[b])) for b in range(B)]
    res = run_bass_kernel_spmd(_NC, in_maps, core_ids=list(range(B)))
    return np.stack([res.results[b]["out"] for b in range(B)], axis=0)


# revision 69
# speedup vs baseline: 1.2160x; 1.1739x over previous
"""Distributed attention kernel for Trainium2 (8 NeuronCores).

Problem: nn_Attention (B=8, S=2048, d_model=512, d_hid=512, fp32).
Sharding: data-parallel over batch - one batch element per core, no
collectives. Each core computes softmax(Q K^T / sqrt(d)) V for its
[2048, 512] slice.

Per-core plan (matmul operands bf16 except where noted):
  * M-trick: scores = (x Wq)(x Wk)^T = x (Wq Wk^T) x^T. Precompute
    M = Wq Wk^T (8k PE cycles) and skip the K projection entirely
    (-32k cycles). qm = M^T x^T plays the role of Q^T; x^T itself
    plays the role of K^T. Bias handling stays exact: the Q-side bias
    terms are constant along the softmax axis (drop), the K-side term
    gamma[k] = x[k] . (Wk bq) is folded into the exp bias.
  * fp8 e4m3 + DoubleRow on half the AV k-chunks: 2x PE throughput on
    those chunks (-33k cycles); denominator computed from the
    quantized P so the correlated part of the quantization error
    cancels. Predicted rel-l2 ~1.75e-2 < 2e-2 gate.
  * Denominator in fp32: DVE tree-sum of the 16 P tiles, then one
    fp32 ones-matmul -> [1, 512]; transposes + DVE reciprocal give
    1/denom per-partition columns.
  * Software pipelining: AV(qb-1) is emitted after scores(qb), so the
    denom chain of a block hides under the next block's scores.
  * DMA with 8KB descriptors: x is loaded 4-consecutive-rows-per-
    partition; the output is assembled likewise by taking stride-4
    stationary slices of P in the AV matmul (o_blk[p, a, :] = row
    4p+a), then written with 4 partition-range dma_starts per block.
  * No max-subtraction in softmax: scores ~ N(0,1).
"""

import sys

for _p in ("/opt/trn_rl_repo",):
    if _p not in sys.path:
        sys.path.append(_p)

from contextlib import ExitStack

import numpy as np

import concourse.bass as bass
import concourse.mybir as mybir
import concourse.tile as tile
from concourse import bacc
from concourse.bass_utils import run_bass_kernel_spmd
from concourse.masks import make_identity

B = 8
S = 2048
D = 512
H = 512
P = 128
NB = 512  # matmul free-dim / PSUM bank (fp32)
FP = mybir.dt.float32
BF = mybir.dt.bfloat16
F8 = mybir.dt.float8e4
DR = mybir.MatmulPerfMode.DoubleRow
SCALE = 1.0 / float(np.sqrt(H))

D_CH = D // P   # 4 contraction chunks
S_T = S // P    # 16 sequence tiles
QB = S // NB    # 4 query blocks
N_BF_K = 4      # k-tiles 0..3 computed in bf16
N_F8_PAIRS = 6  # k-tiles 4..15 as 6 fp8 DoubleRow pairs
EXP = mybir.ActivationFunctionType.Exp
IDENT = mybir.ActivationFunctionType.Identity
# Constant subtracted from logits before exp (softmax-invariant): keeps
# exp(s) well under the e4m3 inf threshold (~240) for s up to ~7.5 sigma.
C_SHIFT = 2.0


DEBUG = False
_DBG = {}


def _build():
    nc = bacc.Bacc("TRN2", target_bir_lowering=False, debug=False)
    x = nc.dram_tensor("x", [S, D], FP, kind="ExternalInput").ap()
    wq = nc.dram_tensor("Wq", [D, H], FP, kind="ExternalInput").ap()
    bq = nc.dram_tensor("bq", [H], FP, kind="ExternalInput").ap()
    wk = nc.dram_tensor("Wk", [D, H], FP, kind="ExternalInput").ap()
    wv = nc.dram_tensor("Wv", [D, H], FP, kind="ExternalInput").ap()
    bv = nc.dram_tensor("bv", [H], FP, kind="ExternalInput").ap()
    out = nc.dram_tensor("out", [S, H], FP, kind="ExternalOutput").ap()
    if DEBUG:
        for nm, shp in (("dbg_xt0", [P, S]), ("dbg_qm0", [P, S]),
                        ("dbg_m0", [P, NB]), ("dbg_p00", [P, NB]),
                        ("dbg_p800", [P, NB]), ("dbg_rc00", [P, 1]),
                        ("dbg_gcol", [P, S_T]), ("dbg_ssum0", [P, NB]),
                        ("dbg_drow0", [1, NB]), ("dbg_drow0v", [1, NB]),
                        ("dbg_xt", [P, 4 * S]), ("dbg_qm", [P, 4 * S]),
                        ("dbg_pall", [P, S_T * NB]),
                        ("dbg_v0", [P, NB]), ("dbg_v80", [P, NB])):
            _DBG[nm] = nc.dram_tensor(nm, shp, FP, kind="ExternalOutput").ap()

    with tile.TileContext(nc, pool_alloc_mode="queue") as tc:
        _body(tc, x, wq, bq, wk, wv, bv, out)
    nc.compile()
    return nc


_DBG_N = [0]


def _dbg_dump(tc, name, src_ap, shape, dst=None):
    if not DEBUG:
        return
    nc = tc.nc
    _DBG_N[0] += 1
    key = f"{name}_{_DBG_N[0]}" if name else f"anon_{_DBG_N[0]}"
    if dst is None:
        dst = _DBG[name]
    pool = tc.tile_pool(name=f"dbgp_{key}", bufs=1)
    with pool as p:
        t = p.tile(shape, FP, tag=f"dbg_{key}")
        nc.vector.tensor_copy(t[:], src_ap)
        nc.sync.dma_start(dst, t[:])


def _body(tc, x, wq, bq, wk, wv, bv, out):
    nc = tc.nc

    with ExitStack() as ctx:
        const_pool = ctx.enter_context(tc.tile_pool(name="const", bufs=1))
        warm_in = const_pool.tile([P, P], BF, tag="warm_in")
        nc.vector.memset(warm_in[:], 1.0)
        ident_bf = const_pool.tile([P, P], BF, tag="ident_bf")
        make_identity(nc, ident_bf[:])
        ident_f = const_pool.tile([4, 4], FP, tag="ident_f")
        make_identity(nc, ident_f[:])
        ones_row = const_pool.tile([1, P], BF, tag="ones_row")
        nc.vector.memset(ones_row[:], 1.0)
        ones_col = const_pool.tile([P, 1], BF, tag="ones_col")
        nc.vector.memset(ones_col[:], 1.0)

        bias_pool = ctx.enter_context(tc.tile_pool(name="bias", bufs=1))
        big_pool = ctx.enter_context(tc.tile_pool(name="big", bufs=1))
        xT = [big_pool.tile([P, S], BF, tag=f"xT{d}", name=f"xT{d}")
              for d in range(D_CH)]
        qm = [big_pool.tile([P, S], BF, tag=f"qm{d}", name=f"qm{d}")
              for d in range(D_CH)]
        v = [big_pool.tile([P, H], BF, tag=f"v{t}", name=f"v{t}")
             for t in range(N_BF_K)]
        v8 = [big_pool.tile([P, 2, H], F8, tag=f"v8_{i}", name=f"v8_{i}")
              for i in range(N_F8_PAIRS)]
        gcol_all = bias_pool.tile([P, S_T], FP, tag="gcol_all")

        psum_mm = ctx.enter_context(tc.tile_pool(name="pmm", bufs=7,
                                                 space="PSUM"))
        psum_sm = ctx.enter_context(tc.tile_pool(name="psm", bufs=1,
                                                 space="PSUM"))

        xctx = ExitStack()
        wst_pool = xctx.enter_context(tc.tile_pool(name="wst", bufs=1))
        wbf_pool = xctx.enter_context(tc.tile_pool(name="wbf", bufs=1))
        m_pool = xctx.enter_context(tc.tile_pool(name="m", bufs=1))
        xs_pool = xctx.enter_context(tc.tile_pool(name="xs", bufs=1))
        xb_pool = xctx.enter_context(tc.tile_pool(name="xb", bufs=2))

        # ---- DMA issue order = consumption order: x0 first (unblocks
        # transposes), then Wq/Wk (M-make), remaining x, Wv (V-proj), biases.
        xs = [xs_pool.tile([P, 4, NB], FP, tag=f"xs{c}", name=f"xs{c}")
              for c in range(QB)]
        xb_of = {}

        def load_x(c, parts=2):
            xr = x[c * NB:(c + 1) * NB, :]
            n = P // parts
            for i in range(parts):
                nc.sync.dma_start(
                    xs[c][i * n:(i + 1) * n],
                    xr[i * n * 4:(i + 1) * n * 4].rearrange(
                        "(p a) d -> p a d", a=4))

        w_stg = {}

        def load_w(name, ap):
            for c in range(D_CH):
                t = wst_pool.tile([P, H], FP, tag=f"{name}s{c}",
                                  name=f"{name}s{c}")
                nc.sync.dma_start(t[0:64], ap[c * P:c * P + 64, :])
                nc.sync.dma_start(t[64:128], ap[c * P + 64:(c + 1) * P, :])
                w_stg[name, c] = t

        def load_w8(name, ap):
            # 4 consecutive rows per partition -> 8KB descriptors; the
            # strided transpose writes later restore natural d order.
            t = wst_pool.tile([P, 4, NB], FP, tag=f"{name}8s",
                              name=f"{name}8s")
            nc.sync.dma_start(
                t[0:64], ap[0:256].rearrange("(p a) h -> p a h", a=4))
            nc.sync.dma_start(
                t[64:128], ap[256:512].rearrange("(p a) h -> p a h", a=4))
            return t

        load_x(0, parts=4)
        bq_stg = bias_pool.tile([1, H], FP, tag="bq_stg")
        nc.sync.dma_start(bq_stg[:], bq[None, :])
        bv_stg = bias_pool.tile([1, H], FP, tag="bv_stg")
        nc.sync.dma_start(bv_stg[:], bv[None, :])
        wq8_stg = load_w8("wq", wq)
        wk8_stg = load_w8("wk", wk)
        for c in range(1, QB):
            load_x(c)
        load_w("wv", wv)

        # scalar ACT-table preload so the first real copy isn't delayed
        sc_warm = const_pool.tile([1, 4], FP, tag="sc_warm")
        nc.scalar.copy(sc_warm[:], ident_f[0:1, 0:4])

        # ---- warmup: ramp the PE clock while DMAs land ----
        warm_ps = psum_mm.tile([P, P], FP, tag="mm", name="warm_ps")
        with nc.named_scope("warmup"):
            for wi in range(80):
                nc.tensor.matmul(warm_ps[:], warm_in[:], warm_in[:],
                                 start=(wi == 0), stop=(wi == 79))

        def emit_transpose(c):
            with nc.named_scope(f"tp{c}"):
                xb = xb_pool.tile([P, 4, NB], BF, tag="xb")
                xb_of[c] = xb
                for a in range(4):
                    nc.scalar.copy(xb[:, a, :], xs[c][:, a, :])
                # d-outer so xT[d] chunks complete in consumption order
                for d in range(D_CH):
                    for a in range(4):
                        pt = psum_mm.tile([P, NB], FP, tag="mm")
                        pt_bf = pt[:].bitcast(BF)
                        nc.tensor.transpose(
                            pt_bf[:, 0:P], xb[:, a, d * P:(d + 1) * P],
                            ident_bf[:])
                        # sigma layout: col a*128+p holds seq row 4p+a
                        nc.vector.tensor_copy(
                            xT[d][:, c * NB + a * P:c * NB + (a + 1) * P],
                            pt_bf[:, 0:P])

        # ---- casts in DMA-arrival order on the DVE: biases, wq8, wk8 ----
        bq_row = bias_pool.tile([1, H], BF, tag="bq_row")
        nc.vector.tensor_copy(bq_row[:], bq_stg[:])
        bv_row = bias_pool.tile([1, H], BF, tag="bv_row")
        nc.vector.tensor_copy(bv_row[:], bv_stg[:])
        wq8 = wbf_pool.tile([P, 4, NB], BF, tag="wq8")
        nc.scalar.copy(wq8[:], wq8_stg[:])
        wk8 = wbf_pool.tile([P, 4, NB], BF, tag="wk8")
        nc.scalar.copy(wk8[:], wk8_stg[:])

        # x0 transposes: x0 lands before the weights; keep the PE busy
        emit_transpose(0)

        with nc.named_scope("bcast"):
            bv_ps = psum_mm.tile([P, NB], FP, tag="mm", name="bv_ps")
            nc.tensor.matmul(bv_ps[:], ones_row[:], bv_row[:], start=True,
                             stop=True)
            bv_full = bias_pool.tile([P, H], FP, tag="bv_full")
            nc.vector.tensor_copy(bv_full[:], bv_ps[:])

        # ---- M = Wq Wk^T (contract over h): transpose Wq/Wk, then MM ----
        # row d = 4p + a of the 8KB layout -> strided col write, like xT
        wT = {}
        with nc.named_scope("wtrans"):
            for name, src8 in (("wk", wk8), ("wq", wq8)):
                for ct in range(D_CH):  # h-chunk ct
                    t = m_pool.tile([P, H], BF, tag=f"{name}T{ct}",
                                    name=f"{name}T{ct}")
                    wT[name, ct] = t
                for ct in range(D_CH):
                    for a in range(4):
                        pt = psum_mm.tile([P, NB], FP, tag="mm")
                        pt_bf = pt[:].bitcast(BF)
                        nc.tensor.transpose(
                            pt_bf[:, 0:P],
                            src8[:, a, ct * P:(ct + 1) * P],
                            ident_bf[:])
                        dst = wT[name, ct][:, a:H:4]
                        if name == "wk":
                            nc.scalar.copy(dst, pt_bf[:, 0:P])
                        else:
                            nc.vector.tensor_copy(dst, pt_bf[:, 0:P])
        m_t = []
        with nc.named_scope("mmake"):
            for i in range(D_CH):
                ps = psum_mm.tile([P, NB], FP, tag="mm")
                for c in range(D_CH):
                    nc.tensor.matmul(
                        ps[:], wT["wq", c][:, i * P:(i + 1) * P],
                        wT["wk", c][:], start=(c == 0), stop=(c == D_CH - 1))
                mt = m_pool.tile([P, NB], BF, tag=f"m{i}", name=f"m{i}")
                nc.scalar.copy(mt[:], ps[:])
                m_t.append(mt)

        # ---- gamma prep: w_col[d] = (Wk bq) chunk columns, via wkT ----
        with nc.named_scope("gprep"):
            bq_col = []
            for c in range(D_CH):
                pt = psum_mm.tile([P, NB], FP, tag="mm")
                pt_bf = pt[:].bitcast(BF)
                nc.tensor.transpose(pt_bf[:, 0:1],
                                    bq_row[0:1, c * P:(c + 1) * P],
                                    ident_bf[0:1, 0:1])
                t = bias_pool.tile([P, 1], BF, tag=f"bqc{c}", name=f"bqc{c}")
                nc.vector.tensor_copy(t[:], pt_bf[:, 0:1])
                bq_col.append(t)
            wrp = psum_sm.tile([1, NB], FP, tag="sm", name="wrow")
            for c in range(D_CH):
                nc.tensor.matmul(wrp[:], bq_col[c][:], wT["wk", c][:],
                                 start=(c == 0), stop=(c == D_CH - 1))
            w_rowb = bias_pool.tile([1, NB], BF, tag="w_rowb")
            nc.scalar.copy(w_rowb[:], wrp[:])
            wf_ps = psum_mm.tile([P, NB], FP, tag="mm", name="wf_ps")
            nc.tensor.matmul(wf_ps[:], ones_row[:], w_rowb[:], start=True,
                             stop=True)
            w_full = bias_pool.tile([P, NB], BF, tag="w_full")
            nc.vector.tensor_copy(w_full[:], wf_ps[:])

        # ---- per chunk: qm-proj + gamma row; V-proj last (Wv lands last) --
        def emit_qm_g(c):
            ss = slice(c * NB, (c + 1) * NB)
            with nc.named_scope(f"qm{c}"):
                for j in range(D_CH):
                    ps = psum_mm.tile([P, NB], FP, tag="mm")
                    for i in range(D_CH):
                        nc.tensor.matmul(
                            ps[:], m_t[i][:, j * P:(j + 1) * P], xT[i][:, ss],
                            start=(i == 0), stop=(i == D_CH - 1))
                    nc.scalar.copy(qm[j][:, ss], ps[:])
            with nc.named_scope(f"g{c}"):
                # gamma[kt][p] = x[seq 4p+a] . w  (DVE mul + free-reduce;
                # partition p of xb[:, a, :] is exactly k-tile kt part. p)
                xb = xb_of[c]
                for a in range(4):
                    kt = c * 4 + a
                    gt = xb_pool.tile([P, NB], FP, tag="gtmp")
                    nc.gpsimd.tensor_mul(gt[:], xb[:, a, :], w_full[:])
                    rt = xb_pool.tile([P, 1], FP, tag="grt")
                    nc.vector.tensor_reduce(rt[:], gt[:],
                                            axis=mybir.AxisListType.X,
                                            op=mybir.AluOpType.add)
                    nc.vector.tensor_scalar(gcol_all[:, kt:kt + 1], rt[:],
                                            SCALE, -C_SHIFT,
                                            mybir.AluOpType.mult,
                                            mybir.AluOpType.add)

        emit_qm_g(0)
        for c in range(1, QB):
            emit_transpose(c)
            emit_qm_g(c)

        # wv casts here: the DVE reaches them only after the early casts
        w_bf = {}
        for c in range(D_CH):
            t = wbf_pool.tile([P, H], BF, tag=f"wvb{c}", name=f"wvb{c}")
            nc.vector.tensor_copy(t[:], w_stg["wv", c][:])
            w_bf["wv", c] = t

        for c in range(QB):
            with nc.named_scope(f"v{c}"):
                for st in range(c * 4, c * 4 + 4):
                    ts = slice(st * P, (st + 1) * P)
                    ps = psum_mm.tile([P, NB], FP, tag="mm")
                    for d in range(D_CH):
                        nc.tensor.matmul(ps[:], xT[d][:, ts],
                                         w_bf["wv", d][:],
                                         start=(d == 0), stop=(d == D_CH - 1))
                    if st < N_BF_K:
                        nc.vector.tensor_add(v[st][:], ps[:], bv_full[:])
                    else:
                        pi, half = (st - N_BF_K) // 2, (st - N_BF_K) % 2
                        vtmp = xb_pool.tile([P, H], BF, tag="vtmp")
                        nc.vector.tensor_add(vtmp[:], ps[:], bv_full[:])
                        nc.vector.tensor_copy(v8[pi][:, half, :], vtmp[:])

        if DEBUG:
            nc.sync.dma_start(_DBG["dbg_gcol"], gcol_all[:])
        _dbg_dump(tc, "dbg_xt0", xT[0][:], [P, S])
        _dbg_dump(tc, "dbg_qm0", qm[0][:], [P, S])
        if DEBUG:
            xt_r = _DBG["dbg_xt"].rearrange("p (c s) -> p c s", c=4)
            qm_r = _DBG["dbg_qm"].rearrange("p (c s) -> p c s", c=4)
            for d in range(D_CH):
                _dbg_dump(tc, "xtd", xT[d][:], [P, S], dst=xt_r[:, d])
                _dbg_dump(tc, "qmd", qm[d][:], [P, S], dst=qm_r[:, d])
        _dbg_dump(tc, "dbg_m0", m_t[0][:], [P, NB])
        _dbg_dump(tc, "dbg_v0", v[0][:], [P, NB])
        _dbg_dump(tc, "dbg_v80", v8[0][:, 0, :], [P, NB])
        xctx.close()

        # ---- attention, software-pipelined across query blocks ----
        p_pool = ctx.enter_context(tc.tile_pool(name="pp", bufs=2))
        sm_pool = ctx.enter_context(tc.tile_pool(name="sm", bufs=2))
        o_pool = ctx.enter_context(tc.tile_pool(name="o", bufs=2))

        p_t = {}    # (qb, k) -> bf16 tile
        p8_t = {}   # (qb, pair) -> fp8 paired tile
        s_sum = {}  # qb -> fp32 denominator sum tile
        rcols = {}  # qb -> list of 4 [128,1] reciprocal cols (a-order)

        def emit_scores(qb):
            qs = slice(qb * NB, (qb + 1) * NB)
            with nc.named_scope(f"sc{qb}"):
                for k in range(S_T):
                    ks = slice(k * P, (k + 1) * P)
                    ps = psum_mm.tile([P, NB], FP, tag="mm")
                    for d in range(D_CH):
                        nc.tensor.matmul(ps[:], xT[d][:, ks], qm[d][:, qs],
                                         start=(d == 0), stop=(d == D_CH - 1))
                    if k < N_BF_K:
                        t = p_pool.tile([P, NB], BF, tag=f"p{k}",
                                        name=f"p{qb}_{k}")
                        p_t[qb, k] = t
                        nc.scalar.activation(t[:], ps[:], EXP, scale=SCALE,
                                             bias=gcol_all[:, k:k + 1])
                    else:
                        pi, half = (k - N_BF_K) // 2, (k - N_BF_K) % 2
                        if half == 0:
                            p8_t[qb, pi] = p_pool.tile(
                                [P, 2, NB], F8, tag=f"p8_{pi}",
                                name=f"p8_{qb}_{pi}")
                        nc.scalar.activation(p8_t[qb, pi][:, half, :], ps[:],
                                             EXP, scale=SCALE,
                                             bias=gcol_all[:, k:k + 1])
                # fp32 denominator tree over all 16 P tile views
                aps = [p_t[qb, k][:] for k in range(N_BF_K)]
                for pi in range(N_F8_PAIRS):
                    aps += [p8_t[qb, pi][:, 0, :], p8_t[qb, pi][:, 1, :]]
                gs = []
                for g in range(4):
                    t = sm_pool.tile([P, NB], FP, tag=f"gs{g}",
                                     name=f"gs{qb}_{g}")
                    nc.vector.tensor_add(t[:], aps[4 * g], aps[4 * g + 1])
                    nc.vector.tensor_add(t[:], t[:], aps[4 * g + 2])
                    nc.vector.tensor_add(t[:], t[:], aps[4 * g + 3])
                    gs.append(t)
                nc.vector.tensor_add(gs[0][:], gs[0][:], gs[1][:])
                nc.vector.tensor_add(gs[2][:], gs[2][:], gs[3][:])
                nc.vector.tensor_add(gs[0][:], gs[0][:], gs[2][:])
                sb = sm_pool.tile([P, NB], BF, tag="sbf", name=f"sbf{qb}")
                nc.vector.tensor_copy(sb[:], gs[0][:])
                s_sum[qb] = sb

        def emit_denom(qb):
            with nc.named_scope(f"dn{qb}"):
                dps = psum_sm.tile([1, NB], FP, tag="sm", name=f"dps{qb}")
                nc.tensor.matmul(dps[:], ones_col[:], s_sum[qb][:],
                                 start=True, stop=True)
                drow = sm_pool.tile([1, NB], FP, tag="drow", name=f"dr{qb}")
                nc.scalar.copy(drow[:], dps[:])
                if DEBUG and qb == 0:
                    nc.sync.dma_start(_DBG["dbg_drow0"], drow[:])
                rc = sm_pool.tile([P, 4], FP, tag="rc", name=f"rc{qb}")
                for a in range(4):
                    rp = psum_sm.tile([P, 1], FP, tag="sm",
                                      name=f"rtp{qb}_{a}")
                    nc.tensor.transpose(rp[:, 0:1],
                                        drow[0:1, a * P:(a + 1) * P],
                                        ident_f[0:1, 0:1])
                    nc.vector.reciprocal(rc[:, a:a + 1], rp[:, 0:1])
                rcols[qb] = rc

        def emit_av(qb):
            with nc.named_scope(f"av{qb}"):
                o_blk = o_pool.tile([P, 4, NB], FP, tag="ob", name=f"ob{qb}")
                for a in range(4):
                    asl = slice(a * P, (a + 1) * P)
                    ps = psum_mm.tile([P, NB], FP, tag="mm")
                    for k in range(N_BF_K):
                        nc.tensor.matmul(ps[:], p_t[qb, k][:, asl],
                                         v[k][:], start=(k == 0), stop=False)
                    for pi in range(N_F8_PAIRS):
                        nc.tensor.matmul(ps[:], p8_t[qb, pi][:, :, asl],
                                         v8[pi][:], start=False,
                                         stop=(pi == N_F8_PAIRS - 1),
                                         perf_mode=DR)
                    nc.vector.tensor_scalar_mul(o_blk[:, a, :], ps[:],
                                                rcols[qb][:, a:a + 1])
                for r in range(4):
                    q0 = qb * NB + r * P
                    nc.sync.dma_start(
                        out[q0:q0 + P, :].rearrange("(p a) h -> p a h", a=4),
                        o_blk[32 * r:32 * (r + 1)])

        for qb in range(QB):
            emit_scores(qb)
            if qb == 0:
                _dbg_dump(tc, "dbg_p00", p_t[0, 0][:], [P, NB])
                _dbg_dump(tc, "dbg_p800", p8_t[0, 0][:, 0, :], [P, NB])
                _dbg_dump(tc, "dbg_ssum0", s_sum[0][:], [P, NB])
                if DEBUG:
                    pall = _DBG["dbg_pall"].rearrange("p (k n) -> p k n",
                                                      k=S_T)
                    for k in range(N_BF_K):
                        _dbg_dump(tc, "pk", p_t[0, k][:], [P, NB],
                                  dst=pall[:, k])
                    for pi in range(N_F8_PAIRS):
                        for hf in range(2):
                            _dbg_dump(tc, "p8k", p8_t[0, pi][:, hf, :],
                                      [P, NB],
                                      dst=pall[:, N_BF_K + 2 * pi + hf])
            if qb > 0:
                emit_denom(qb - 1)
                if qb == 1:
                    _dbg_dump(tc, "dbg_rc00", rcols[0][:, 0:1], [P, 1])
                emit_av(qb - 1)
        emit_denom(QB - 1)
        emit_av(QB - 1)


_NC = None


def kernel(**inputs):
    global _NC
    if _NC is None:
        _NC = _build()
    x = np.ascontiguousarray(np.asarray(inputs["x"], dtype=np.float32))
    shared = {
        k: np.ascontiguousarray(np.asarray(inputs[k], dtype=np.float32))
        for k in ("Wq", "bq", "Wk", "Wv", "bv")
    }
    in_maps = [dict(shared, x=np.ascontiguousarray(x[b])) for b in range(B)]
    res = run_bass_kernel_spmd(_NC, in_maps, core_ids=list(range(B)))
    return np.stack([res.results[b]["out"] for b in range(B)], axis=0)


# revision 70
# speedup vs baseline: 1.2487x; 1.0269x over previous
"""Distributed attention kernel for Trainium2 (8 NeuronCores).

Problem: nn_Attention (B=8, S=2048, d_model=512, d_hid=512, fp32).
Sharding: data-parallel over batch - one batch element per core, no
collectives. Each core computes softmax(Q K^T / sqrt(d)) V for its
[2048, 512] slice.

Per-core plan (matmul operands bf16 except where noted):
  * M-trick: scores = (x Wq)(x Wk)^T = x (Wq Wk^T) x^T. Precompute
    M = Wq Wk^T (8k PE cycles) and skip the K projection entirely
    (-32k cycles). qm = M^T x^T plays the role of Q^T; x^T itself
    plays the role of K^T. Bias handling stays exact: the Q-side bias
    terms are constant along the softmax axis (drop), the K-side term
    gamma[k] = x[k] . (Wk bq) is folded into the exp bias.
  * fp8 e4m3 + DoubleRow on half the AV k-chunks: 2x PE throughput on
    those chunks (-33k cycles); denominator computed from the
    quantized P so the correlated part of the quantization error
    cancels. Predicted rel-l2 ~1.75e-2 < 2e-2 gate.
  * Denominator in fp32: DVE tree-sum of the 16 P tiles, then one
    fp32 ones-matmul -> [1, 512]; transposes + DVE reciprocal give
    1/denom per-partition columns.
  * Software pipelining: AV(qb-1) is emitted after scores(qb), so the
    denom chain of a block hides under the next block's scores.
  * DMA with 8KB descriptors: x is loaded 4-consecutive-rows-per-
    partition; the output is assembled likewise by taking stride-4
    stationary slices of P in the AV matmul (o_blk[p, a, :] = row
    4p+a), then written with 4 partition-range dma_starts per block.
  * No max-subtraction in softmax: scores ~ N(0,1).
"""

import sys

for _p in ("/opt/trn_rl_repo",):
    if _p not in sys.path:
        sys.path.append(_p)

from contextlib import ExitStack

import numpy as np

import concourse.bass as bass
import concourse.mybir as mybir
import concourse.tile as tile
from concourse import bacc
from concourse.bass_utils import run_bass_kernel_spmd
from concourse.masks import make_identity

B = 8
S = 2048
D = 512
H = 512
P = 128
NB = 512  # matmul free-dim / PSUM bank (fp32)
FP = mybir.dt.float32
BF = mybir.dt.bfloat16
F8 = mybir.dt.float8e4
DR = mybir.MatmulPerfMode.DoubleRow
SCALE = 1.0 / float(np.sqrt(H))

D_CH = D // P   # 4 contraction chunks
S_T = S // P    # 16 sequence tiles
QB = S // NB    # 4 query blocks
N_BF_K = 4      # k-tiles 0..3 computed in bf16
N_F8_PAIRS = 6  # k-tiles 4..15 as 6 fp8 DoubleRow pairs
EXP = mybir.ActivationFunctionType.Exp
IDENT = mybir.ActivationFunctionType.Identity
# Constant subtracted from logits before exp (softmax-invariant): keeps
# exp(s) well under the e4m3 inf threshold (~240) for s up to ~7.5 sigma.
C_SHIFT = 2.0


DEBUG = False
_DBG = {}


def _build():
    nc = bacc.Bacc("TRN2", target_bir_lowering=False, debug=False)
    x = nc.dram_tensor("x", [S, D], FP, kind="ExternalInput").ap()
    wq = nc.dram_tensor("Wq", [D, H], FP, kind="ExternalInput").ap()
    bq = nc.dram_tensor("bq", [H], FP, kind="ExternalInput").ap()
    wk = nc.dram_tensor("Wk", [D, H], FP, kind="ExternalInput").ap()
    wv = nc.dram_tensor("Wv", [D, H], FP, kind="ExternalInput").ap()
    bv = nc.dram_tensor("bv", [H], FP, kind="ExternalInput").ap()
    out = nc.dram_tensor("out", [S, H], FP, kind="ExternalOutput").ap()
    if DEBUG:
        for nm, shp in (("dbg_xt0", [P, S]), ("dbg_qm0", [P, S]),
                        ("dbg_m0", [P, NB]), ("dbg_p00", [P, NB]),
                        ("dbg_p800", [P, NB]), ("dbg_rc00", [P, 1]),
                        ("dbg_gcol", [P, S_T]), ("dbg_ssum0", [P, NB]),
                        ("dbg_drow0", [1, NB]), ("dbg_drow0v", [1, NB]),
                        ("dbg_xt", [P, 4 * S]), ("dbg_qm", [P, 4 * S]),
                        ("dbg_pall", [P, S_T * NB]),
                        ("dbg_v0", [P, NB]), ("dbg_v80", [P, NB])):
            _DBG[nm] = nc.dram_tensor(nm, shp, FP, kind="ExternalOutput").ap()

    with tile.TileContext(nc, pool_alloc_mode="queue") as tc:
        _body(tc, x, wq, bq, wk, wv, bv, out)
    nc.compile()
    return nc


_DBG_N = [0]


def _dbg_dump(tc, name, src_ap, shape, dst=None):
    if not DEBUG:
        return
    nc = tc.nc
    _DBG_N[0] += 1
    key = f"{name}_{_DBG_N[0]}" if name else f"anon_{_DBG_N[0]}"
    if dst is None:
        dst = _DBG[name]
    pool = tc.tile_pool(name=f"dbgp_{key}", bufs=1)
    with pool as p:
        t = p.tile(shape, FP, tag=f"dbg_{key}")
        nc.vector.tensor_copy(t[:], src_ap)
        nc.sync.dma_start(dst, t[:])


def _body(tc, x, wq, bq, wk, wv, bv, out):
    nc = tc.nc

    with ExitStack() as ctx:
        const_pool = ctx.enter_context(tc.tile_pool(name="const", bufs=1))
        warm_in = const_pool.tile([P, P], BF, tag="warm_in")
        nc.vector.memset(warm_in[:], 1.0)
        ident_bf = const_pool.tile([P, P], BF, tag="ident_bf")
        make_identity(nc, ident_bf[:])
        ident_f = const_pool.tile([4, 4], FP, tag="ident_f")
        make_identity(nc, ident_f[:])
        ones_row = const_pool.tile([1, P], BF, tag="ones_row")
        nc.vector.memset(ones_row[:], 1.0)
        ones_col = const_pool.tile([P, 1], BF, tag="ones_col")
        nc.vector.memset(ones_col[:], 1.0)

        bias_pool = ctx.enter_context(tc.tile_pool(name="bias", bufs=1))
        big_pool = ctx.enter_context(tc.tile_pool(name="big", bufs=1))
        xT = [big_pool.tile([P, S], BF, tag=f"xT{d}", name=f"xT{d}")
              for d in range(D_CH)]
        qm = [big_pool.tile([P, S], BF, tag=f"qm{d}", name=f"qm{d}")
              for d in range(D_CH)]
        v = [big_pool.tile([P, H], BF, tag=f"v{t}", name=f"v{t}")
             for t in range(N_BF_K)]
        v8 = [big_pool.tile([P, 2, H], F8, tag=f"v8_{i}", name=f"v8_{i}")
              for i in range(N_F8_PAIRS)]
        gcol_all = bias_pool.tile([P, S_T], FP, tag="gcol_all")

        psum_mm = ctx.enter_context(tc.tile_pool(name="pmm", bufs=7,
                                                 space="PSUM"))
        psum_sm = ctx.enter_context(tc.tile_pool(name="psm", bufs=1,
                                                 space="PSUM"))

        xctx = ExitStack()
        wst_pool = xctx.enter_context(tc.tile_pool(name="wst", bufs=1))
        wbf_pool = xctx.enter_context(tc.tile_pool(name="wbf", bufs=1))
        m_pool = xctx.enter_context(tc.tile_pool(name="m", bufs=1))
        xs_pool = xctx.enter_context(tc.tile_pool(name="xs", bufs=1))
        xb_pool = xctx.enter_context(tc.tile_pool(name="xb", bufs=2))

        # ---- DMA issue order = consumption order: x0 first (unblocks
        # transposes), then Wq/Wk (M-make), remaining x, Wv (V-proj), biases.
        xs = [xs_pool.tile([P, 4, NB], FP, tag=f"xs{c}", name=f"xs{c}")
              for c in range(QB)]
        xb_of = {}

        def load_x(c, parts=2):
            xr = x[c * NB:(c + 1) * NB, :]
            n = P // parts
            for i in range(parts):
                nc.sync.dma_start(
                    xs[c][i * n:(i + 1) * n],
                    xr[i * n * 4:(i + 1) * n * 4].rearrange(
                        "(p a) d -> p a d", a=4))

        w_stg = {}

        def load_w(name, ap):
            for c in range(D_CH):
                t = wst_pool.tile([P, H], FP, tag=f"{name}s{c}",
                                  name=f"{name}s{c}")
                nc.sync.dma_start(t[0:64], ap[c * P:c * P + 64, :])
                nc.sync.dma_start(t[64:128], ap[c * P + 64:(c + 1) * P, :])
                w_stg[name, c] = t

        def load_w8(name, ap):
            # 4 consecutive rows per partition -> 8KB descriptors; the
            # strided transpose writes later restore natural d order.
            t = wst_pool.tile([P, 4, NB], FP, tag=f"{name}8s",
                              name=f"{name}8s")
            nc.sync.dma_start(
                t[0:64], ap[0:256].rearrange("(p a) h -> p a h", a=4))
            nc.sync.dma_start(
                t[64:128], ap[256:512].rearrange("(p a) h -> p a h", a=4))
            return t

        load_x(0, parts=4)
        bq_stg = bias_pool.tile([1, H], FP, tag="bq_stg")
        nc.sync.dma_start(bq_stg[:], bq[None, :])
        bv_stg = bias_pool.tile([1, H], FP, tag="bv_stg")
        nc.sync.dma_start(bv_stg[:], bv[None, :])
        wq8_stg = load_w8("wq", wq)
        wk8_stg = load_w8("wk", wk)
        for c in range(1, QB):
            load_x(c)
        load_w("wv", wv)

        # scalar ACT-table preload so the first real copy isn't delayed
        sc_warm = const_pool.tile([1, 4], FP, tag="sc_warm")
        nc.scalar.copy(sc_warm[:], ident_f[0:1, 0:4])

        # ---- warmup: ramp the PE clock while DMAs land ----
        warm_ps = psum_mm.tile([P, P], FP, tag="mm", name="warm_ps")
        with nc.named_scope("warmup"):
            for wi in range(80):
                nc.tensor.matmul(warm_ps[:], warm_in[:], warm_in[:],
                                 start=(wi == 0), stop=(wi == 79))

        def emit_transpose(c):
            with nc.named_scope(f"tp{c}"):
                xb = xb_pool.tile([P, 4, NB], BF, tag="xb")
                xb_of[c] = xb
                for a in range(4):
                    nc.scalar.copy(xb[:, a, :], xs[c][:, a, :])
                # d-outer so xT[d] chunks complete in consumption order
                for d in range(D_CH):
                    for a in range(4):
                        pt = psum_mm.tile([P, NB], FP, tag="mm")
                        pt_bf = pt[:].bitcast(BF)
                        nc.tensor.transpose(
                            pt_bf[:, 0:P], xb[:, a, d * P:(d + 1) * P],
                            ident_bf[:])
                        # sigma layout: col a*128+p holds seq row 4p+a
                        nc.vector.tensor_copy(
                            xT[d][:, c * NB + a * P:c * NB + (a + 1) * P],
                            pt_bf[:, 0:P])

        # ---- casts in DMA-arrival order on the DVE: biases, wq8, wk8 ----
        bq_row = bias_pool.tile([1, H], BF, tag="bq_row")
        nc.vector.tensor_copy(bq_row[:], bq_stg[:])
        bv_row = bias_pool.tile([1, H], BF, tag="bv_row")
        nc.vector.tensor_copy(bv_row[:], bv_stg[:])
        wq8 = wbf_pool.tile([P, 4, NB], BF, tag="wq8")
        nc.scalar.copy(wq8[:], wq8_stg[:])
        wk8 = wbf_pool.tile([P, 4, NB], BF, tag="wk8")
        nc.scalar.copy(wk8[:], wk8_stg[:])

        # x0 transposes: x0 lands before the weights; keep the PE busy
        emit_transpose(0)

        with nc.named_scope("bcast"):
            bv_ps = psum_mm.tile([P, NB], FP, tag="mm", name="bv_ps")
            nc.tensor.matmul(bv_ps[:], ones_row[:], bv_row[:], start=True,
                             stop=True)
            bv_full = bias_pool.tile([P, H], FP, tag="bv_full")
            nc.vector.tensor_copy(bv_full[:], bv_ps[:])

        # ---- M = Wq Wk^T (contract over h): transpose Wq/Wk, then MM ----
        # row d = 4p + a of the 8KB layout -> strided col write, like xT
        wT = {}
        with nc.named_scope("wtrans"):
            for name, src8 in (("wq", wq8), ("wk", wk8)):
                for ct in range(D_CH):  # h-chunk ct
                    t = m_pool.tile([P, H], BF, tag=f"{name}T{ct}",
                                    name=f"{name}T{ct}")
                    wT[name, ct] = t
                for ct in range(D_CH):
                    for a in range(4):
                        pt = psum_mm.tile([P, NB], FP, tag="mm")
                        pt_bf = pt[:].bitcast(BF)
                        nc.tensor.transpose(
                            pt_bf[:, 0:P],
                            src8[:, a, ct * P:(ct + 1) * P],
                            ident_bf[:])
                        dst = wT[name, ct][:, a:H:4]
                        if name == "wk":
                            nc.scalar.copy(dst, pt_bf[:, 0:P])
                        else:
                            nc.vector.tensor_copy(dst, pt_bf[:, 0:P])
        m_t = []
        with nc.named_scope("mmake"):
            for i in range(D_CH):
                ps = psum_mm.tile([P, NB], FP, tag="mm")
                for c in range(D_CH):
                    nc.tensor.matmul(
                        ps[:], wT["wq", c][:, i * P:(i + 1) * P],
                        wT["wk", c][:], start=(c == 0), stop=(c == D_CH - 1))
                mt = m_pool.tile([P, NB], BF, tag=f"m{i}", name=f"m{i}")
                nc.scalar.copy(mt[:], ps[:])
                m_t.append(mt)

        # ---- gamma prep: w_col[d] = (Wk bq) chunk columns, via wkT ----
        with nc.named_scope("gprep"):
            bq_col = []
            for c in range(D_CH):
                pt = psum_mm.tile([P, NB], FP, tag="mm")
                pt_bf = pt[:].bitcast(BF)
                nc.tensor.transpose(pt_bf[:, 0:1],
                                    bq_row[0:1, c * P:(c + 1) * P],
                                    ident_bf[0:1, 0:1])
                t = bias_pool.tile([P, 1], BF, tag=f"bqc{c}", name=f"bqc{c}")
                nc.vector.tensor_copy(t[:], pt_bf[:, 0:1])
                bq_col.append(t)
            wrp = psum_sm.tile([1, NB], FP, tag="sm", name="wrow")
            for c in range(D_CH):
                nc.tensor.matmul(wrp[:], bq_col[c][:], wT["wk", c][:],
                                 start=(c == 0), stop=(c == D_CH - 1))
            w_rowb = bias_pool.tile([1, NB], BF, tag="w_rowb")
            nc.scalar.copy(w_rowb[:], wrp[:])
            wf_ps = psum_mm.tile([P, NB], FP, tag="mm", name="wf_ps")
            nc.tensor.matmul(wf_ps[:], ones_row[:], w_rowb[:], start=True,
                             stop=True)
            w_full = bias_pool.tile([P, NB], BF, tag="w_full")
            nc.vector.tensor_copy(w_full[:], wf_ps[:])

        # ---- per chunk: qm-proj + gamma row; V-proj last (Wv lands last) --
        def emit_qm_g(c):
            ss = slice(c * NB, (c + 1) * NB)
            with nc.named_scope(f"qm{c}"):
                for j in range(D_CH):
                    ps = psum_mm.tile([P, NB], FP, tag="mm")
                    for i in range(D_CH):
                        nc.tensor.matmul(
                            ps[:], m_t[i][:, j * P:(j + 1) * P], xT[i][:, ss],
                            start=(i == 0), stop=(i == D_CH - 1))
                    nc.scalar.copy(qm[j][:, ss], ps[:])
            with nc.named_scope(f"g{c}"):
                # gamma[kt][p] = x[seq 4p+a] . w  (DVE mul + free-reduce;
                # partition p of xb[:, a, :] is exactly k-tile kt part. p)
                xb = xb_of[c]
                for a in range(4):
                    kt = c * 4 + a
                    gt = xb_pool.tile([P, NB], FP, tag="gtmp")
                    nc.gpsimd.tensor_mul(gt[:], xb[:, a, :], w_full[:])
                    rt = xb_pool.tile([P, 1], FP, tag="grt")
                    nc.vector.tensor_reduce(rt[:], gt[:],
                                            axis=mybir.AxisListType.X,
                                            op=mybir.AluOpType.add)
                    nc.vector.tensor_scalar(gcol_all[:, kt:kt + 1], rt[:],
                                            SCALE, -C_SHIFT,
                                            mybir.AluOpType.mult,
                                            mybir.AluOpType.add)

        emit_qm_g(0)
        for c in range(1, QB):
            emit_transpose(c)
            emit_qm_g(c)

        # wv casts here: the DVE reaches them only after the early casts
        w_bf = {}
        for c in range(D_CH):
            t = wbf_pool.tile([P, H], BF, tag=f"wvb{c}", name=f"wvb{c}")
            nc.vector.tensor_copy(t[:], w_stg["wv", c][:])
            w_bf["wv", c] = t

        for c in range(QB):
            with nc.named_scope(f"v{c}"):
                for st in range(c * 4, c * 4 + 4):
                    ts = slice(st * P, (st + 1) * P)
                    ps = psum_mm.tile([P, NB], FP, tag="mm")
                    for d in range(D_CH):
                        nc.tensor.matmul(ps[:], xT[d][:, ts],
                                         w_bf["wv", d][:],
                                         start=(d == 0), stop=(d == D_CH - 1))
                    if st < N_BF_K:
                        nc.vector.tensor_add(v[st][:], ps[:], bv_full[:])
                    else:
                        pi, half = (st - N_BF_K) // 2, (st - N_BF_K) % 2
                        vtmp = xb_pool.tile([P, H], BF, tag="vtmp")
                        nc.vector.tensor_add(vtmp[:], ps[:], bv_full[:])
                        nc.vector.tensor_copy(v8[pi][:, half, :], vtmp[:])

        if DEBUG:
            nc.sync.dma_start(_DBG["dbg_gcol"], gcol_all[:])
        _dbg_dump(tc, "dbg_xt0", xT[0][:], [P, S])
        _dbg_dump(tc, "dbg_qm0", qm[0][:], [P, S])
        if DEBUG:
            xt_r = _DBG["dbg_xt"].rearrange("p (c s) -> p c s", c=4)
            qm_r = _DBG["dbg_qm"].rearrange("p (c s) -> p c s", c=4)
            for d in range(D_CH):
                _dbg_dump(tc, "xtd", xT[d][:], [P, S], dst=xt_r[:, d])
                _dbg_dump(tc, "qmd", qm[d][:], [P, S], dst=qm_r[:, d])
        _dbg_dump(tc, "dbg_m0", m_t[0][:], [P, NB])
        _dbg_dump(tc, "dbg_v0", v[0][:], [P, NB])
        _dbg_dump(tc, "dbg_v80", v8[0][:, 0, :], [P, NB])
        xctx.close()

        # ---- attention, software-pipelined across query blocks ----
        p_pool = ctx.enter_context(tc.tile_pool(name="pp", bufs=2))
        sm_pool = ctx.enter_context(tc.tile_pool(name="sm", bufs=2))
        o_pool = ctx.enter_context(tc.tile_pool(name="o", bufs=2))

        p_t = {}    # (qb, k) -> bf16 tile
        p8_t = {}   # (qb, pair) -> fp8 paired tile
        s_sum = {}  # qb -> fp32 denominator sum tile
        rcols = {}  # qb -> list of 4 [128,1] reciprocal cols (a-order)

        def emit_scores(qb):
            qs = slice(qb * NB, (qb + 1) * NB)
            with nc.named_scope(f"sc{qb}"):
                for k in range(S_T):
                    ks = slice(k * P, (k + 1) * P)
                    ps = psum_mm.tile([P, NB], FP, tag="mm")
                    for d in range(D_CH):
                        nc.tensor.matmul(ps[:], xT[d][:, ks], qm[d][:, qs],
                                         start=(d == 0), stop=(d == D_CH - 1))
                    if k < N_BF_K:
                        t = p_pool.tile([P, NB], BF, tag=f"p{k}",
                                        name=f"p{qb}_{k}")
                        p_t[qb, k] = t
                        nc.scalar.activation(t[:], ps[:], EXP, scale=SCALE,
                                             bias=gcol_all[:, k:k + 1])
                    else:
                        pi, half = (k - N_BF_K) // 2, (k - N_BF_K) % 2
                        if half == 0:
                            p8_t[qb, pi] = p_pool.tile(
                                [P, 2, NB], F8, tag=f"p8_{pi}",
                                name=f"p8_{qb}_{pi}")
                        nc.scalar.activation(p8_t[qb, pi][:, half, :], ps[:],
                                             EXP, scale=SCALE,
                                             bias=gcol_all[:, k:k + 1])
                # fp32 denominator tree over all 16 P tile views
                aps = [p_t[qb, k][:] for k in range(N_BF_K)]
                for pi in range(N_F8_PAIRS):
                    aps += [p8_t[qb, pi][:, 0, :], p8_t[qb, pi][:, 1, :]]
                gs = []
                for g in range(4):
                    t = sm_pool.tile([P, NB], FP, tag=f"gs{g}",
                                     name=f"gs{qb}_{g}")
                    nc.vector.tensor_add(t[:], aps[4 * g], aps[4 * g + 1])
                    nc.vector.tensor_add(t[:], t[:], aps[4 * g + 2])
                    nc.vector.tensor_add(t[:], t[:], aps[4 * g + 3])
                    gs.append(t)
                nc.vector.tensor_add(gs[0][:], gs[0][:], gs[1][:])
                nc.vector.tensor_add(gs[2][:], gs[2][:], gs[3][:])
                nc.vector.tensor_add(gs[0][:], gs[0][:], gs[2][:])
                sb = sm_pool.tile([P, NB], BF, tag="sbf", name=f"sbf{qb}")
                nc.vector.tensor_copy(sb[:], gs[0][:])
                s_sum[qb] = sb

        def emit_denom(qb):
            with nc.named_scope(f"dn{qb}"):
                dps = psum_sm.tile([1, NB], FP, tag="sm", name=f"dps{qb}")
                nc.tensor.matmul(dps[:], ones_col[:], s_sum[qb][:],
                                 start=True, stop=True)
                drow = sm_pool.tile([1, NB], FP, tag="drow", name=f"dr{qb}")
                nc.scalar.copy(drow[:], dps[:])
                if DEBUG and qb == 0:
                    nc.sync.dma_start(_DBG["dbg_drow0"], drow[:])
                rc = sm_pool.tile([P, 4], FP, tag="rc", name=f"rc{qb}")
                for a in range(4):
                    rp = psum_sm.tile([P, 1], FP, tag="sm",
                                      name=f"rtp{qb}_{a}")
                    nc.tensor.transpose(rp[:, 0:1],
                                        drow[0:1, a * P:(a + 1) * P],
                                        ident_f[0:1, 0:1])
                    nc.vector.reciprocal(rc[:, a:a + 1], rp[:, 0:1])
                rcols[qb] = rc

        def emit_av(qb):
            with nc.named_scope(f"av{qb}"):
                o_blk = o_pool.tile([P, 4, NB], FP, tag="ob", name=f"ob{qb}")
                for a in range(4):
                    asl = slice(a * P, (a + 1) * P)
                    ps = psum_mm.tile([P, NB], FP, tag="mm")
                    for k in range(N_BF_K):
                        nc.tensor.matmul(ps[:], p_t[qb, k][:, asl],
                                         v[k][:], start=(k == 0), stop=False)
                    for pi in range(N_F8_PAIRS):
                        nc.tensor.matmul(ps[:], p8_t[qb, pi][:, :, asl],
                                         v8[pi][:], start=False,
                                         stop=(pi == N_F8_PAIRS - 1),
                                         perf_mode=DR)
                    nc.vector.tensor_scalar_mul(o_blk[:, a, :], ps[:],
                                                rcols[qb][:, a:a + 1])
                for r in range(4):
                    q0 = qb * NB + r * P
                    nc.sync.dma_start(
                        out[q0:q0 + P, :].rearrange("(p a) h -> p a h", a=4),
                        o_blk[32 * r:32 * (r + 1)])

        for qb in range(QB):
            emit_scores(qb)
            if qb == 0:
                _dbg_dump(tc, "dbg_p00", p_t[0, 0][:], [P, NB])
                _dbg_dump(tc, "dbg_p800", p8_t[0, 0][:, 0, :], [P, NB])
                _dbg_dump(tc, "dbg_ssum0", s_sum[0][:], [P, NB])
                if DEBUG:
                    pall = _DBG["dbg_pall"].rearrange("p (k n) -> p k n",
                                                      k=S_T)
                    for k in range(N_BF_K):
                        _dbg_dump(tc, "pk", p_t[0, k][:], [P, NB],
                                  dst=pall[:, k])
                    for pi in range(N_F8_PAIRS):
                        for hf in range(2):
                            _dbg_dump(tc, "p8k", p8_t[0, pi][:, hf, :],
                                      [P, NB],
                                      dst=pall[:, N_BF_K + 2 * pi + hf])
            if qb > 0:
                emit_denom(qb - 1)
                if qb == 1:
                    _dbg_dump(tc, "dbg_rc00", rcols[0][:, 0:1], [P, 1])
                emit_av(qb - 1)
        emit_denom(QB - 1)
        emit_av(QB - 1)


_NC = None


def kernel(**inputs):
    global _NC
    if _NC is None:
        _NC = _build()
    x = np.ascontiguousarray(np.asarray(inputs["x"], dtype=np.float32))
    shared = {
        k: np.ascontiguousarray(np.asarray(inputs[k], dtype=np.float32))
        for k in ("Wq", "bq", "Wk", "Wv", "bv")
    }
    in_maps = [dict(shared, x=np.ascontiguousarray(x[b])) for b in range(B)]
    res = run_bass_kernel_spmd(_NC, in_maps, core_ids=list(range(B)))
    return np.stack([res.results[b]["out"] for b in range(B)], axis=0)
